# revision 1
# baseline (speedup 1.0000x reference)
"""GCNEncoder (GCNConv + TransformerEncoderLayer) on 8 Trainium2 NeuronCores.

Sharding: nodes are split 512/core (8 cores). Per core:
  - GCN: dense normalized-adjacency block A^T [4096 src, 512 dst] built on
    device via GPSIMD local_scatter from host-permuted (index-only) edge
    layouts; aggregation is a dense fp16 matmul against the AllGathered
    scaled features.
  - Attention: both heads, q = the core's 512 nodes vs all 4096 keys.
    Scores computed transposed (S^T[k,q]) so softmax denominators come from
    a ones-matmul and PV needs no transposes; softmax skips max-subtraction
    (scores are O(1) for this model family; exp cannot overflow fp32).
  - FFN + both LayerNorms fully local.
Two AllGathers (scaled GCN features, hidden-state transpose) are the only
collectives. All matmul operands fp16, accumulation fp32 in PSUM.
"""

import math

import numpy as np

import concourse.bacc as bacc
import concourse.mybir as mybir
import concourse.tile as tile
from concourse import library_config
from concourse.tile_rust import add_dep_helper

N_CORES = 8
N = 4096
E = 131072
DIN = 512
D = 256
H = 2
DH = 128
DFF = 2048
EPS = 1e-5
P = 128

NPC = N // N_CORES          # nodes per core = 512
MPC = NPC // P              # m-chunks per core = 4
KT = N // P                 # src k-tiles = 32
KPAD = 32                   # max out-edges per (core, src-node)
KBD = 80                    # max in-edges per dst node
NDUP = 256                  # max duplicate-edge occurrences per core
DT16 = mybir.dt.float16
DT32 = mybir.dt.float32
DTI16 = mybir.dt.int16
F = mybir.ActivationFunctionType
A = mybir.AluOpType
INV_SQRT_DH = 1.0 / math.sqrt(DH)


def build_kernel():
    nc = bacc.Bacc("TRN2", target_bir_lowering=False, debug=False,
                   num_devices=N_CORES)

    def din(name, shape, dt=DT32):
        return nc.dram_tensor(name, shape, dt, kind="ExternalInput")

    xT_d = din("xT", [P, MPC * DIN], DT16)
    xTf_d = din("xTf", [P, (DIN // P) * N], DT16)   # full x.T wrapped
    wbdf_d = din("wbdf", [P, (N // P) * KBD], DT16)  # full per-dst weights
    wg_d = din("wg", [P, (DIN // P) * D], DT16)
    warr_d = din("warr", [P, KT * KPAD], DT16)
    idx_d = din("idx", [P, KT * KPAD], DTI16)
    wbd_d = din("wbd", [P, MPC * KBD], DT16)
    dupsr_d = din("dupsr", [P, NDUP // P])
    dupfc_d = din("dupfc", [P, NDUP // P])
    dupw_d = din("dupw", [P, NDUP // P])
    iota1024_d = din("iota1024", [P, KT * KPAD])
    iota128_d = din("iota128", [P, P])
    ident_d = din("ident", [P, P])
    winT_d = din("winT", [P, 2 * 3 * D], DT16)
    ipb_d = din("ipb", [P, 6])
    woT_d = din("woT", [P, 2 * D], DT16)
    w1T_d = din("w1T", [P, 2 * DFF], DT16)
    b1_d = din("b1", [P, DFF // P])
    w2T_d = din("w2T", [P, (DFF // P) * D], DT16)
    bias_d = din("bias", [1, 7 * D])

    out_d = nc.dram_tensor("out", [NPC, D], DT32, kind="ExternalOutput")

    with tile.TileContext(nc) as tc:
        with (
            tc.tile_pool(name="keep", bufs=1) as keep,
            tc.tile_pool(name="dram", bufs=1, space="DRAM") as dram,
        ):
            def load16(dram_t, cols):
                f16 = keep.tile([P, cols], DT16, tag=f"ld_{dram_t.name}",
                                name=f"{dram_t.name}16")
                nc.sync.dma_start(f16[:], dram_t[:])
                return f16

            def bc4(ap_2d):
                """[128, D] bias slice -> broadcast [128, MPC, D]."""
                return ap_2d[:, None, :].to_broadcast([P, MPC, D])

            ones16_col = keep.tile([P, 1], DT16)
            ones16_row = keep.tile([1, P], DT16)
            ones32_row = keep.tile([1, P], DT32)
            nc.vector.memset(ones16_col[:], 1.0)
            nc.vector.memset(ones16_row[:], 1.0)
            nc.vector.memset(ones32_row[:], 1.0)

            lib = nc.gpsimd.load_library(library_config.local_scatter)

            gk = ctx_gcn = tc.tile_pool(name="gcn_keep", bufs=1)
            gk = ctx_gcn.__enter__()

            # ---- A build first: scatters on GpSimd start ASAP ----
            iota1024 = gk.tile([P, KT * KPAD], DT32)
            iota128 = gk.tile([P, P], DT32)
            warr = gk.tile([P, KT * KPAD], DT16)
            idx_t = gk.tile([P, KT * KPAD], DTI16)
            dupsr = gk.tile([P, NDUP // P], DT32)
            dupfc = gk.tile([P, NDUP // P], DT32)
            dupw = gk.tile([P, NDUP // P], DT32)
            nc.sync.dma_start(warr[:], warr_d[:])
            nc.sync.dma_start(idx_t[:], idx_d[:])
            nc.sync.dma_start(iota1024[:], iota1024_d[:])
            nc.sync.dma_start(iota128[:], iota128_d[:])
            nc.sync.dma_start(dupsr[:], dupsr_d[:])
            nc.sync.dma_start(dupfc[:], dupfc_d[:])
            nc.sync.dma_start(dupw[:], dupw_d[:])

            warr16 = gk.tile([P, KT * KPAD], DT16)
            a_tiles = [gk.tile([P, NPC], DT16, tag=f"A{kt}", name=f"A{kt}")
                       for kt in range(KT)]

            with tc.tile_pool(name="gcn_sb", bufs=2) as gsb, \
                 tc.tile_pool(name="gcn_ps", bufs=2, space="PSUM") as gps:
                mrg_ps = [gps.tile([P, 512], DT32, space="PSUM",
                                   tag=f"mrg{h}", name=f"mrg{h}")
                          for h in range(2)]
                for b in range(NDUP // P):
                    sd = gsb.tile([P, P], DT16, tag="sd")
                    vd = gsb.tile([P, KT * KPAD], DT16, tag="vd")
                    nc.vector.tensor_scalar(sd[:], iota128[:],
                                            dupsr[:, b:b + 1], None,
                                            op0=A.is_equal)
                    nc.vector.tensor_scalar(vd[:], iota1024[:],
                                            dupfc[:, b:b + 1],
                                            dupw[:, b:b + 1],
                                            op0=A.is_equal, op1=A.mult)
                    for h in range(2):
                        nc.tensor.matmul(mrg_ps[h][:], lhsT=sd[:],
                                         rhs=vd[:, 512 * h:512 * h + 512],
                                         start=(b == 0),
                                         stop=(b == NDUP // P - 1))
                for h in range(2):
                    nc.vector.tensor_tensor(warr16[:, 512 * h:512 * h + 512],
                                            warr[:, 512 * h:512 * h + 512],
                                            mrg_ps[h][:], op=A.add)
                last_scatter = None
                for kt in range(KT):
                    ls = nc.gpsimd.local_scatter(
                        a_tiles[kt][:],
                        warr16[:, KPAD * kt:KPAD * (kt + 1)],
                        idx_t[:, KPAD * kt:KPAD * (kt + 1)],
                        channels=P, num_elems=NPC, num_idxs=KPAD,
                    )
                    add_dep_helper(ls.ins, lib.ins, reason="scatter after lib")
                    last_scatter = ls

            # ---- degrees -> dinv (local + full) ----
            wbd = gk.tile([P, MPC * KBD], DT16)
            nc.sync.dma_start(wbd[:], wbd_d[:])
            dinv = gk.tile([P, MPC], DT32)
            dinv2 = gk.tile([P, MPC], DT32)
            deg = gk.tile([P, MPC], DT32)
            nc.vector.tensor_reduce(
                deg[:], wbd[:].rearrange("p (m k) -> p m k", k=KBD),
                axis=mybir.AxisListType.X, op=A.add)
            sqd = gk.tile([P, MPC], DT32)
            nc.scalar.activation(sqd[:], deg[:], F.Sqrt, bias=1.0, scale=1.0)
            nc.vector.reciprocal(dinv[:], sqd[:])
            nc.vector.tensor_mul(dinv2[:], dinv[:], dinv[:])

            wbdf = gk.tile([P, (N // P) * KBD], DT16)
            nc.sync.dma_start(wbdf[:], wbdf_d[:])
            dinvf = gk.tile([P, N // P], DT32)
            degf = gk.tile([P, N // P], DT32)
            nc.vector.tensor_reduce(
                degf[:], wbdf[:].rearrange("p (j k) -> p j k", k=KBD),
                axis=mybir.AxisListType.X, op=A.add)
            sqdf = gk.tile([P, N // P], DT32)
            nc.scalar.activation(sqdf[:], degf[:], F.Sqrt, bias=1.0, scale=1.0)
            nc.vector.reciprocal(dinvf[:], sqdf[:])

            # ---- xw = x @ W_gcn: full (replicated) + local self-term ----
            xT16 = load16(xT_d, MPC * DIN)
            wg16 = load16(wg_d, (DIN // P) * D)
            xTf16 = gk.tile([P, (DIN // P) * N], DT16)
            nc.sync.dma_start(xTf16[:], xTf_d[:])
            xws16f = gk.tile([P, (N // P) * D], DT16)
            self32 = gk.tile([P, MPC * D], DT32)
            with tc.tile_pool(name="xw_ps", bufs=4, space="PSUM") as xps:
                for m in range(MPC):
                    pxw = xps.tile([P, D], DT32, space="PSUM", tag="xw")
                    for k in range(DIN // P):
                        nc.tensor.matmul(
                            pxw[:],
                            lhsT=xT16[:, DIN * k + P * m:DIN * k + P * m + P],
                            rhs=wg16[:, D * k:D * (k + 1)],
                            start=(k == 0), stop=(k == DIN // P - 1))
                    nc.vector.tensor_scalar(self32[:, D * m:D * (m + 1)], pxw[:],
                                            dinv2[:, m:m + 1], None, op0=A.mult)
                for j in range(N // P):
                    pxw = xps.tile([P, D], DT32, space="PSUM", tag="xw")
                    for k in range(DIN // P):
                        nc.tensor.matmul(
                            pxw[:],
                            lhsT=xTf16[:, N * k + P * j:N * k + P * (j + 1)],
                            rhs=wg16[:, D * k:D * (k + 1)],
                            start=(k == 0), stop=(k == DIN // P - 1))
                    nc.vector.tensor_scalar(xws16f[:, D * j:D * (j + 1)],
                                            pxw[:], dinvf[:, j:j + 1], None,
                                            op0=A.mult)

            # constants for later phases (DMA after critical ones)
            ident = keep.tile([P, P], DT32)
            ipb = keep.tile([P, 6], DT32)
            b1t = keep.tile([P, DFF // P], DT32)
            nc.sync.dma_start(ident[:], ident_d[:])
            nc.sync.dma_start(ipb[:], ipb_d[:])
            nc.sync.dma_start(b1t[:], b1_d[:])
            winT16 = load16(winT_d, 2 * 3 * D)
            woT16 = load16(woT_d, 2 * D)

            bias_row = keep.tile([1, 7 * D], DT32)
            nc.sync.dma_start(bias_row[:], bias_d[:])
            ipb16 = keep.tile([P, 6], DT16)
            nc.vector.tensor_copy(ipb16[:], ipb[:])
            bias_bc = keep.tile([P, 7 * D], DT32)
            with tc.tile_pool(name="ps_b", bufs=2, space="PSUM") as psb:
                for j in range(4):
                    w = 448 if j < 3 else 7 * D - 3 * 448
                    pb = psb.tile([P, 448], DT32, space="PSUM", tag="bb")
                    nc.tensor.matmul(pb[:, :w], lhsT=ones32_row[:],
                                     rhs=bias_row[:, j * 448:j * 448 + w],
                                     start=True, stop=(j != 3))
                    if j == 3:
                        # softmax rows sum to 1, so the V bias contributes the
                        # constant (concat_h bv_h) @ W_o^T — accumulate it
                        # onto out_proj_b in the broadcast tile.
                        for h in range(H):
                            nc.tensor.matmul(
                                pb[:, 192:448],
                                lhsT=ipb16[:, 4 + h:5 + h].to_broadcast([P, P]),
                                rhs=woT16[:, D * h:D * (h + 1)],
                                start=False, stop=(h == H - 1))
                    nc.vector.tensor_copy(bias_bc[:, j * 448:j * 448 + w],
                                          pb[:, :w])
            bgcn_bc = bias_bc[:, 0:D]
            b2_bc = bias_bc[:, D:2 * D]
            ln1g_bc = bias_bc[:, 2 * D:3 * D]
            ln1b_bc = bias_bc[:, 3 * D:4 * D]
            ln2g_bc = bias_bc[:, 4 * D:5 * D]
            ln2b_bc = bias_bc[:, 5 * D:6 * D]
            bo_bc = bias_bc[:, 6 * D:7 * D]

            # ---- aggregation ----
            h_t = keep.tile([P, MPC * D], DT32)
            hT16 = keep.tile([P, 2 * NPC], DT16)
            with tc.tile_pool(name="agg_sb", bufs=1) as asb, \
                 tc.tile_pool(name="agg_ps", bufs=1, space="PSUM") as aps:
                agg_ps = [aps.tile([P, D], DT32, space="PSUM",
                                   tag=f"agg{m}", name=f"agg{m}")
                          for m in range(MPC)]
                for kt in range(KT):
                    for m in range(MPC):
                        agg_mm = nc.tensor.matmul(
                            agg_ps[m][:],
                            lhsT=a_tiles[kt][:, P * m:P * (m + 1)],
                            rhs=xws16f[:, D * kt:D * (kt + 1)],
                            start=(kt == 0), stop=(kt == KT - 1))
                        if kt == 0:
                            # single barrier: stream all 128 agg matmuls after
                            # the last scatter instead of trickling per-tile
                            add_dep_helper(agg_mm.ins, last_scatter.ins,
                                           reason="agg after all scatters")

                # h = relu(dinv*agg + self + b_gcn)   (batched epilogue)
                x_all = asb.tile([P, MPC * D], DT32, tag="xall")
                for m in range(MPC):
                    nc.vector.scalar_tensor_tensor(
                        x_all[:, D * m:D * (m + 1)], agg_ps[m][:],
                        dinv[:, m:m + 1], self32[:, D * m:D * (m + 1)],
                        op0=A.mult, op1=A.add)
                nc.vector.tensor_tensor(
                    x_all[:].rearrange("p (m d) -> p m d", m=MPC),
                    x_all[:].rearrange("p (m d) -> p m d", m=MPC),
                    bc4(bgcn_bc), op=A.add)
                nc.scalar.activation(h_t[:], x_all[:], F.Relu)

            # transpose h -> hT16 (local feature-major)
            with tc.tile_pool(name="tr_ps", bufs=2, space="PSUM") as tps:
                for m in range(MPC):
                    for f in range(2):
                        ptr = tps.tile([P, P], DT32, space="PSUM", tag="tr")
                        nc.tensor.transpose(
                            ptr[:], h_t[:, D * m + P * f:D * m + P * (f + 1)],
                            ident[:])
                        nc.vector.tensor_copy(
                            hT16[:, NPC * f + P * m:NPC * f + P * (m + 1)],
                            ptr[:])

            ctx_gcn.__exit__(None, None, None)
            ak = ctx_attn = tc.tile_pool(name="attn_keep", bufs=1)
            ak = ctx_attn.__enter__()

            # ---- local K^T / V / Q^T, then ONE packed KV AllGather ----
            # kv rows: 0:128 K^T h0 | 128:256 K^T h1 | 256:384 V h0 | 384:512 V h1
            # (V packed as [128, m*128+d] = natural [512, 128] per head)
            qT16 = ak.tile([P, H * NPC], DT16)
            kv_sb = ak.tile([P, 4 * NPC], DT16)
            with tc.tile_pool(name="kv_ps", bufs=3, space="PSUM") as kvps:
                for h in range(H):
                    pq = kvps.tile([P, NPC], DT32, space="PSUM", tag="kv")
                    for k in range(2):
                        nc.tensor.matmul(
                            pq[:],
                            lhsT=winT16[:, 768 * k + P * h:768 * k + P * (h + 1)],
                            rhs=hT16[:, NPC * k:NPC * (k + 1)],
                            start=(k == 0), stop=(k == 1))
                    nc.vector.tensor_scalar(
                        qT16[:, NPC * h:NPC * (h + 1)], pq[:],
                        ipb[:, h:h + 1], None, op0=A.add)
                    pk = kvps.tile([P, NPC], DT32, space="PSUM", tag="kv")
                    for k in range(2):
                        nc.tensor.matmul(
                            pk[:],
                            lhsT=winT16[:, 768 * k + D + P * h:
                                        768 * k + D + P * (h + 1)],
                            rhs=hT16[:, NPC * k:NPC * (k + 1)],
                            start=(k == 0), stop=(k == 1))
                    nc.vector.tensor_scalar(
                        kv_sb[:, NPC * h:NPC * (h + 1)], pk[:],
                        ipb[:, 2 + h:3 + h], None, op0=A.add)
                    for m in range(MPC):
                        pv = kvps.tile([P, P], DT32, space="PSUM", tag="kvv")
                        for k in range(2):
                            nc.tensor.matmul(
                                pv[:],
                                lhsT=hT16[:, NPC * k + P * m:NPC * k + P * (m + 1)],
                                rhs=winT16[:, 768 * k + 2 * D + P * h:
                                            768 * k + 2 * D + P * (h + 1)],
                                start=(k == 0), stop=(k == 1))
                        nc.vector.tensor_copy(
                            kv_sb[:, NPC * (2 + h) + P * m:
                                  NPC * (2 + h) + P * (m + 1)], pv[:])

            # FFN weights stream before/while the AllGather runs
            w1T16 = ak.tile([P, 2 * DFF], DT16)
            nc.sync.dma_start(w1T16[:], w1T_d[:])
            w2T16 = ak.tile([P, (DFF // P) * D], DT16)
            nc.sync.dma_start(w2T16[:], w2T_d[:])

            kv_bounce = dram.tile([4 * P, NPC], DT16)
            kv_gath = dram.tile([N_CORES * 4 * P, NPC], DT16,
                                addr_space="Shared")
            nc.scalar.dma_start(
                kv_bounce[:].rearrange("(x p) n -> p x n", p=P),
                kv_sb[:].rearrange("p (x n) -> p x n", x=4))
            nc.gpsimd.collective_compute(
                "AllGather", A.bypass,
                replica_groups=[list(range(N_CORES))],
                ins=[kv_bounce.opt()], outs=[kv_gath.opt()])

            # ---- load gathered K^T / V ----
            kT16 = ak.tile([P, H * N], DT16)
            v16 = ak.tile([P, H * N], DT16)
            gv = kv_gath[:].rearrange("(g x p) n -> x p g n",
                                      g=N_CORES, x=4, p=P)
            for h in range(H):
                nc.scalar.dma_start(
                    kT16[:, N * h:N * (h + 1)].rearrange(
                        "p (g n) -> p g n", g=N_CORES), gv[h])
                nc.scalar.dma_start(
                    v16[:, N * h:N * (h + 1)].rearrange(
                        "p (g n) -> p g n", g=N_CORES), gv[2 + h])

            # ---- S^T -> exp -> PV + sums ----
            oT16 = ak.tile([P, H * NPC], DT16)
            with tc.tile_pool(name="att_sb", bufs=3) as atsb, \
                 tc.tile_pool(name="att_ps", bufs=1, space="PSUM") as atps, \
                 tc.tile_pool(name="s_ps", bufs=2, space="PSUM") as sps:
                o_ps = [atps.tile([P, NPC], DT32, space="PSUM",
                                  tag=f"o{h}", name=f"o{h}")
                        for h in range(H)]
                sum_ps = [atps.tile([1, NPC], DT32, space="PSUM",
                                    tag=f"sm{h}", name=f"sm{h}")
                          for h in range(H)]
                esum = [None, None]
                for kt2 in range(KT // 2):
                    for h in range(H):
                        # two k-tiles of scores into one 2-bank psum; one exp
                        ps_s = sps.tile([P, 2 * NPC], DT32, space="PSUM",
                                        tag="S")
                        for u in range(2):
                            kt = 2 * kt2 + u
                            nc.tensor.matmul(
                                ps_s[:, NPC * u:NPC * (u + 1)],
                                lhsT=kT16[:, N * h + P * kt:N * h + P * (kt + 1)],
                                rhs=qT16[:, NPC * h:NPC * (h + 1)],
                                start=True, stop=True)
                        es = atsb.tile([P, 2 * NPC], DT16, tag="es")
                        nc.scalar.activation(es[:], ps_s[:], F.Exp,
                                             scale=INV_SQRT_DH)
                        for u in range(2):
                            kt = 2 * kt2 + u
                            nc.tensor.matmul(
                                o_ps[h][:],
                                lhsT=v16[:, N * h + P * kt:N * h + P * (kt + 1)],
                                rhs=es[:, NPC * u:NPC * (u + 1)],
                                start=(kt == 0), stop=(kt == KT - 1))
                        if kt2 % 2 == 0:
                            eacc = atsb.tile([P, 2 * NPC], DT16, tag=f"eac{h}",
                                             name=f"eacc{h}")
                            nc.vector.tensor_copy(eacc[:], es[:])
                            esum[h] = eacc
                        else:
                            nc.vector.tensor_add(esum[h][:], esum[h][:], es[:])
                            for u in range(2):
                                nc.tensor.matmul(
                                    sum_ps[h][:], lhsT=ones16_col[:],
                                    rhs=esum[h][:, NPC * u:NPC * (u + 1)],
                                    start=(kt2 == 1 and u == 0),
                                    stop=(kt2 == KT // 2 - 1 and u == 1))

                # copy unnormalized o to sbuf; transpose sums to
                # per-partition [128, MPC] reciprocals
                recT = atsb.tile([P, H * MPC], DT32, tag="recT", name="recT")
                for h in range(H):
                    nc.vector.tensor_copy(oT16[:, NPC * h:NPC * (h + 1)],
                                          o_ps[h][:])
                    srow = atsb.tile([1, NPC], DT32, tag="srow")
                    nc.vector.tensor_copy(srow[:], sum_ps[h][:])
                    sT_ps = sps.tile([P, MPC], DT32, space="PSUM", tag="S",
                                     name="sTps")
                    for m in range(MPC):
                        nc.tensor.transpose(
                            sT_ps[:, m:m + 1], srow[:, P * m:P * (m + 1)],
                            ident[0:1, 0:1])
                    nc.vector.reciprocal(recT[:, MPC * h:MPC * (h + 1)],
                                         sT_ps[:])

            # ---- o_proj + residual + LN1 (batched) ----
            h1_t = ak.tile([P, MPC * D], DT32)
            h1T16 = ak.tile([P, 2 * NPC], DT16)
            with tc.tile_pool(name="ln_sb", bufs=2) as lsb, \
                 tc.tile_pool(name="op_ps", bufs=2, space="PSUM") as ops:

                def layernorm_all(dst, x_all, g_sl, b_sl, tag):
                    """LN over feature dim for all MPC chunks at once.
                    x_all/dst: [128, MPC*D] fp32 tiles."""
                    mu4 = lsb.tile([P, MPC], DT32, tag=f"{tag}mu")
                    nc.vector.tensor_reduce(
                        mu4[:], x_all[:].rearrange("p (m d) -> p m d", m=MPC),
                        axis=mybir.AxisListType.X, op=A.add)
                    negmu4 = lsb.tile([P, MPC], DT32, tag=f"{tag}nm")
                    nc.vector.tensor_scalar(negmu4[:], mu4[:], -1.0 / D, None,
                                            op0=A.mult)
                    sq4 = lsb.tile([P, D], DT32, tag=f"{tag}sq")
                    ssq4 = lsb.tile([P, MPC], DT32, tag=f"{tag}ss")
                    for m in range(MPC):
                        nc.scalar.activation(sq4[:], x_all[:, D * m:D * (m + 1)],
                                             F.Square, bias=negmu4[:, m:m + 1],
                                             accum_out=ssq4[:, m:m + 1])
                    var4 = lsb.tile([P, MPC], DT32, tag=f"{tag}vr")
                    nc.vector.tensor_scalar(var4[:], ssq4[:], 1.0 / D, EPS,
                                            op0=A.mult, op1=A.add)
                    sd4 = lsb.tile([P, MPC], DT32, tag=f"{tag}sd")
                    nc.scalar.activation(sd4[:], var4[:], F.Sqrt)
                    rstd4 = lsb.tile([P, MPC], DT32, tag=f"{tag}rs")
                    nc.vector.reciprocal(rstd4[:], sd4[:])
                    xc = lsb.tile([P, MPC * D], DT32, tag=f"{tag}xc")
                    for m in range(MPC):
                        nc.vector.tensor_scalar(
                            xc[:, D * m:D * (m + 1)], x_all[:, D * m:D * (m + 1)],
                            negmu4[:, m:m + 1], rstd4[:, m:m + 1],
                            op0=A.add, op1=A.mult)
                    nc.vector.tensor_tensor(
                        xc[:].rearrange("p (m d) -> p m d", m=MPC),
                        xc[:].rearrange("p (m d) -> p m d", m=MPC),
                        bc4(g_sl), op=A.mult)
                    nc.vector.tensor_tensor(
                        dst[:].rearrange("p (m d) -> p m d", m=MPC),
                        xc[:].rearrange("p (m d) -> p m d", m=MPC),
                        bc4(b_sl), op=A.add)

                x1_all = lsb.tile([P, MPC * D], DT32, tag="x1all")
                for m in range(MPC):
                    pa = [None, None]
                    for h in range(H):
                        pa[h] = ops.tile([P, D], DT32, space="PSUM", tag="op", name=f"pa{h}")
                        nc.tensor.matmul(
                            pa[h][:],
                            lhsT=oT16[:, NPC * h + P * m:NPC * h + P * (m + 1)],
                            rhs=woT16[:, D * h:D * (h + 1)],
                            start=True, stop=True)
                    t0m = lsb.tile([P, D], DT32, tag="t0m")
                    nc.vector.tensor_scalar(t0m[:], pa[0][:],
                                            recT[:, m:m + 1], None,
                                            op0=A.mult)
                    nc.vector.scalar_tensor_tensor(
                        t0m[:], pa[1][:], recT[:, MPC + m:MPC + m + 1],
                        t0m[:], op0=A.mult, op1=A.add)
                    nc.vector.tensor_add(x1_all[:, D * m:D * (m + 1)], t0m[:],
                                         h_t[:, D * m:D * (m + 1)])
                nc.vector.tensor_tensor(
                    x1_all[:].rearrange("p (m d) -> p m d", m=MPC),
                    x1_all[:].rearrange("p (m d) -> p m d", m=MPC),
                    bc4(bo_bc), op=A.add)
                layernorm_all(h1_t, x1_all, ln1g_bc, ln1b_bc, "a")

                with tc.tile_pool(name="tr2_ps", bufs=2, space="PSUM") as tps2:
                    for m in range(MPC):
                        for f in range(2):
                            ptr = tps2.tile([P, P], DT32, space="PSUM",
                                            tag="tr2")
                            nc.tensor.transpose(
                                ptr[:],
                                h1_t[:, D * m + P * f:D * m + P * (f + 1)],
                                ident[:])
                            nc.vector.tensor_copy(
                                h1T16[:, NPC * f + P * m:NPC * f + P * (m + 1)],
                                ptr[:])

                # ---- FFN ----
                out_sb = ak.tile([P, MPC * D], DT32)
                ff1T = ak.tile([P, (DFF // P) * NPC], DT16)
                with tc.tile_pool(name="f1_ps", bufs=3, space="PSUM") as fps:
                    for dc in range(DFF // P):
                        pf = fps.tile([P, NPC], DT32, space="PSUM", tag="f1")
                        for k in range(2):
                            nc.tensor.matmul(
                                pf[:],
                                lhsT=w1T16[:, DFF * k + P * dc:
                                           DFF * k + P * (dc + 1)],
                                rhs=h1T16[:, NPC * k:NPC * (k + 1)],
                                start=(k == 0), stop=(k == 1))
                        nc.scalar.activation(
                            ff1T[:, NPC * dc:NPC * (dc + 1)], pf[:], F.Relu,
                            bias=b1t[:, dc:dc + 1])

                x2_all = lsb.tile([P, MPC * D], DT32, tag="x2all")
                with tc.tile_pool(name="f2_ps", bufs=2, space="PSUM") as fps2:
                    for m in range(MPC):
                        pf2 = fps2.tile([P, D], DT32, space="PSUM", tag="f2")
                        for kt2 in range(DFF // P):
                            nc.tensor.matmul(
                                pf2[:],
                                lhsT=ff1T[:, NPC * kt2 + P * m:
                                          NPC * kt2 + P * (m + 1)],
                                rhs=w2T16[:, D * kt2:D * (kt2 + 1)],
                                start=(kt2 == 0), stop=(kt2 == DFF // P - 1))
                        nc.vector.scalar_tensor_tensor(
                            x2_all[:, D * m:D * (m + 1)], pf2[:], 1.0,
                            h1_t[:, D * m:D * (m + 1)], op0=A.mult, op1=A.add)
                nc.vector.tensor_tensor(
                    x2_all[:].rearrange("p (m d) -> p m d", m=MPC),
                    x2_all[:].rearrange("p (m d) -> p m d", m=MPC),
                    bc4(b2_bc), op=A.add)
                layernorm_all(out_sb, x2_all, ln2g_bc, ln2b_bc, "b")
                nc.scalar.dma_start(
                    out_d[:].rearrange("(m p) d -> p m d", p=P),
                    out_sb[:].rearrange("p (m d) -> p m d", m=MPC))
            ctx_attn.__exit__(None, None, None)

    nc.compile()
    return nc


# ======================= host-side prep =======================

def _prep_inputs(x, edge_index, edge_weight, W_gcn, b_gcn, in_proj_w,
                 in_proj_b, out_proj_w, out_proj_b, lin1_w, lin1_b, lin2_w,
                 lin2_b, ln1_g, ln1_b, ln2_g, ln2_b):
    """Pure index-permutation / layout prep. Returns per-core input maps."""
    x = np.asarray(x, np.float32)
    src = np.asarray(edge_index[0], np.int64)
    dst = np.asarray(edge_index[1], np.int64)
    w = np.asarray(edge_weight, np.float32)

    def wrap128(a):
        # [n*128, m] -> [128, n*m] with col block t <- rows [128t, 128t+128)
        n = a.shape[0] // P
        return np.ascontiguousarray(
            a.reshape(n, P, a.shape[1]).transpose(1, 0, 2).reshape(P, -1))

    iota1024 = np.tile(np.arange(KT * KPAD, dtype=np.float32), (P, 1))
    iota128 = np.tile(np.arange(P, dtype=np.float32), (P, 1))
    ident = np.eye(P, dtype=np.float32)
    bias_stack = np.concatenate([
        np.asarray(v, np.float32).reshape(-1) for v in
        (b_gcn, lin2_b, ln1_g, ln1_b, ln2_g, ln2_b, out_proj_b)
    ]).reshape(1, -1)

    f16 = np.float16
    shared = {
        "wg": wrap128(np.asarray(W_gcn, np.float32)).astype(f16),
        "iota1024": iota1024, "iota128": iota128,
        "ident": ident,
        "winT": wrap128(np.ascontiguousarray(
            np.asarray(in_proj_w, np.float32).T)).astype(f16),
        "ipb": np.ascontiguousarray(
            np.asarray(in_proj_b, np.float32).reshape(6, P).T),
        "woT": wrap128(np.ascontiguousarray(
            np.asarray(out_proj_w, np.float32).T)).astype(f16),
        "w1T": wrap128(np.ascontiguousarray(
            np.asarray(lin1_w, np.float32).T)).astype(f16),
        "b1": np.ascontiguousarray(
            np.asarray(lin1_b, np.float32).reshape(DFF // P, P).T),
        "w2T": wrap128(np.ascontiguousarray(
            np.asarray(lin2_w, np.float32).T)).astype(f16),
        "bias": bias_stack,
    }

    shared_xTf = wrap128(np.ascontiguousarray(x.T)).astype(f16)
    # full per-dst weight table for replicated degree computation
    wbdf = np.zeros((N, KBD), np.float32)
    cntf = np.zeros(N, np.int32)
    for di, wi in zip(dst.tolist(), w.tolist()):
        j = int(cntf[di])
        assert j < KBD
        wbdf[di, j] = wi
        cntf[di] = j + 1
    wbdf_full_w = wrap128(wbdf).astype(f16)

    core_of = dst // NPC
    in_maps = []
    for c in range(N_CORES):
        sel = np.nonzero(core_of == c)[0]
        s_c = src[sel]
        d_c = (dst[sel] - NPC * c).astype(np.int64)
        w_c = w[sel]

        w_arr = np.zeros((N, KPAD), np.float32)
        idx_arr = np.full((N, KPAD), -1, np.int16)
        counts = np.zeros(N, np.int32)
        first_slot = {}
        dup_sr, dup_fc, dup_w = [], [], []
        for si, di, wi in zip(s_c.tolist(), d_c.tolist(), w_c.tolist()):
            key = si * NPC + di
            slot = first_slot.get(key)
            if slot is None:
                j = int(counts[si])
                assert j < KPAD, f"KPAD overflow at src {si}"
                counts[si] = j + 1
                w_arr[si, j] = wi
                idx_arr[si, j] = di
                first_slot[key] = j
            else:
                dup_sr.append(si % P)
                dup_fc.append(KPAD * (si // P) + slot)
                dup_w.append(wi)
        assert len(dup_sr) <= NDUP, f"NDUP overflow: {len(dup_sr)}"

        def pad_dup(vals, dtype=np.float32):
            a = np.zeros(NDUP, dtype)
            a[:len(vals)] = vals
            return np.ascontiguousarray(a.reshape(NDUP // P, P).T)

        wbd = np.zeros((NPC, KBD), np.float32)
        cnt2 = np.zeros(NPC, np.int32)
        for di, wi in zip(d_c.tolist(), w_c.tolist()):
            j = int(cnt2[di])
            assert j < KBD, f"KBD overflow at dst {di}"
            wbd[di, j] = wi
            cnt2[di] = j + 1

        in_maps.append({
            **shared,
            "xT": wrap128(np.ascontiguousarray(
                x[NPC * c:NPC * (c + 1)].T)).astype(f16),
            "xTf": shared_xTf,
            "wbdf": wbdf_full_w,
            "warr": wrap128(w_arr).astype(f16),
            "idx": wrap128(idx_arr),
            "wbd": wrap128(wbd).astype(f16),
            "dupsr": pad_dup(dup_sr),
            "dupfc": pad_dup(dup_fc),
            "dupw": pad_dup(dup_w),
        })
    return in_maps


# ======================= runner =======================

class _Runner:
    """Persistent-jit SPMD executor (mirrors bass2jax.run_bass_via_pjrt)."""

    def __init__(self, nc):
        import jax
        from jax.sharding import Mesh, PartitionSpec
        from jax.experimental.shard_map import shard_map
        from concourse.bass2jax import (_bass_exec_p, install_neuronx_cc_hook,
                                        partition_id_tensor)
        install_neuronx_cc_hook()
        self.jax = jax
        partition_name = (nc.partition_id_tensor.name
                          if nc.partition_id_tensor else None)
        in_names, out_names, out_avals, zero_outs = [], [], [], []
        for alloc in nc.m.functions[0].allocations:
            if not isinstance(alloc, mybir.MemoryLocationSet):
                continue
            name = alloc.memorylocations[0].name
            if alloc.kind == "ExternalInput":
                if name != partition_name:
                    in_names.append(name)
            elif alloc.kind == "ExternalOutput":
                out_names.append(name)
                shape = tuple(alloc.tensor_shape)
                dtype = mybir.dt.np(alloc.dtype)
                out_avals.append(jax.core.ShapedArray(shape, dtype))
                zero_outs.append(np.zeros(shape, dtype))
        self.in_names, self.out_names = in_names, out_names
        self.out_shapes = [tuple(a.shape) for a in out_avals]
        self.n_params = len(in_names)
        self.zero_outs = zero_outs
        all_in = in_names + out_names
        if partition_name is not None:
            all_in.append(partition_name)

        def _body(*args):
            operands = list(args)
            if partition_name is not None:
                operands.append(partition_id_tensor())
            return tuple(_bass_exec_p.bind(
                *operands, out_avals=tuple(out_avals), in_names=tuple(all_in),
                out_names=tuple(out_names), lowering_input_output_aliases=(),
                sim_require_finite=True, sim_require_nnan=True, nc=nc))

        devices = jax.devices()[:N_CORES]
        self.mesh = Mesh(np.asarray(devices), ("core",))
        nin = self.n_params + len(out_names)
        self.fn = jax.jit(
            shard_map(_body, mesh=self.mesh,
                      in_specs=(PartitionSpec("core"),) * nin,
                      out_specs=(PartitionSpec("core"),) * len(out_names),
                      check_rep=False),
            keep_unused=True)

    def place(self, in_maps):
        import jax
        from jax.sharding import PartitionSpec
        per_core = [[np.asarray(m[n]) for n in self.in_names] for m in in_maps]
        concat = [np.concatenate([per_core[c][i] for c in range(N_CORES)], axis=0)
                  for i in range(self.n_params)]
        zeros = [np.zeros((N_CORES * z.shape[0], *z.shape[1:]), z.dtype)
                 for z in self.zero_outs]
        sh = jax.sharding.NamedSharding(self.mesh, PartitionSpec("core"))
        return [jax.device_put(a, sh) for a in (*concat, *zeros)]

    def run(self, args):
        outs = self.fn(*args)
        self.jax.block_until_ready(outs)
        return outs

    def results(self, outs):
        res = []
        for c in range(N_CORES):
            d = {}
            for i, name in enumerate(self.out_names):
                full = np.asarray(outs[i])
                ps = self.out_shapes[i]
                d[name] = full.reshape((N_CORES,) + ps)[c]
            res.append(d)
        return res


_CACHE = {}


def _get_runner():
    if "runner" not in _CACHE:
        nc = build_kernel()
        _CACHE["nc"] = nc
        _CACHE["runner"] = _Runner(nc)
    return _CACHE["runner"]


def kernel(**inputs) -> np.ndarray:
    runner = _get_runner()
    in_maps = _prep_inputs(**inputs)
    args = runner.place(in_maps)
    outs = runner.run(args)
    res = runner.results(outs)
    return np.concatenate([res[c]["out"] for c in range(N_CORES)], axis=0)



# revision 12
# speedup vs baseline: 1.1377x; 1.1377x over previous
"""GCNEncoder (GCNConv + TransformerEncoderLayer) on 8 Trainium2 NeuronCores.

Sharding: nodes split 512/core. Per core:
  - GCN: dense normalized-adjacency blocks A [4096 src, 512 dst] built on
    device via GPSIMD local_scatter from host-prenormalized edge values
    (deg/dinv/dup-merge/self-loops folded in at host); aggregation runs
    TRANSPOSED (h^T = xw^T-stationary @ A) so h lands feature-major with no
    transposes anywhere in the pipeline.
  - Attention in fp8: q/K^T/V cast to fp8e4 (x16 scaled), ONE 2MB fp8
    AllGather of K/V, per-rank chunked gather loads so scores start early.
    exp probs kept fp8; PV and the softmax denominators use fp8 DoubleRow
    matmuls (2x, contraction pairs are the natural tile layout).
  - Post-attention stays transposed: o scaled by reciprocal-row broadcast,
    out_proj^T, LayerNorms via ones-matmul row reductions, FFN fp16 with
    ff2^T, final LN2^T written transposed; host un-transposes the output.
"""

import math

import numpy as np

import concourse.bacc as bacc
import concourse.mybir as mybir
import concourse.tile as tile
from concourse import library_config
from concourse.tile_rust import add_dep_helper

N_CORES = 8
N = 4096
E = 131072
DIN = 512
D = 256
H = 2
DH = 128
DFF = 2048
EPS = 1e-5
P = 128

NPC = N // N_CORES          # nodes per core = 512
MPC = NPC // P              # m-chunks per core = 4
KT = N // P                 # src k-tiles = 32
KPAD = 32                   # max out-edges per (core, src-node)
FSC = 16.0                  # fp8 pre-scale for q/k/v
DT8 = mybir.dt.float8e4
DT16 = mybir.dt.float16
DT32 = mybir.dt.float32
DTI16 = mybir.dt.int16
F = mybir.ActivationFunctionType
A = mybir.AluOpType
DR = mybir.MatmulPerfMode.DoubleRow
INV_SQRT_DH = 1.0 / math.sqrt(DH)
EXP_SCALE = INV_SQRT_DH / (FSC * FSC)


def build_kernel():
    nc = bacc.Bacc("TRN2", target_bir_lowering=False, debug=False,
                   num_devices=N_CORES)

    def din(name, shape, dt=DT32):
        return nc.dram_tensor(name, shape, dt, kind="ExternalInput")

    warr_d = din("warr", [P, KT * KPAD], DT16)
    idx_d = din("idx", [P, KT * KPAD], DTI16)
    xTf_d = din("xTf", [P, (DIN // P) * N], DT16)   # full x.T wrapped
    wg_d = din("wg", [P, (DIN // P) * D], DT16)
    winT_d = din("winT", [P, 2 * 3 * D], DT16)
    ipb_d = din("ipb", [P, 6])
    woT2_d = din("woT2", [P, 4 * P], DT16)
    w1T_d = din("w1T", [P, 2 * DFF], DT16)
    b1_d = din("b1", [P, DFF // P])
    w2T2_d = din("w2T2", [P, (DFF // P) * D], DT16)
    cols_d = din("cols", [P, 14])

    out_d = nc.dram_tensor("out", [P, 2 * NPC], DT32, kind="ExternalOutput")

    with tile.TileContext(nc) as tc:
        with (
            tc.tile_pool(name="keep", bufs=1) as keep,
            tc.tile_pool(name="dram", bufs=1, space="DRAM") as dram,
        ):
            ones8 = keep.tile([P, 32], DT8)
            ones16c = keep.tile([P, 1], DT16)
            ones16r = keep.tile([1, P], DT16)
            eps1 = keep.tile([1, 1], DT32)
            nc.vector.memset(ones8[:], 1.0)
            nc.vector.memset(ones16c[:], 1.0)
            nc.vector.memset(ones16r[:], 1.0)
            nc.vector.memset(eps1[:], EPS)

            lib = nc.gpsimd.load_library(library_config.local_scatter)

            gk = ctx_gcn = tc.tile_pool(name="gcn_keep", bufs=1)
            gk = ctx_gcn.__enter__()

            # ---- A-build inputs first: scatters on GpSimd start ASAP ----
            warr = gk.tile([P, KT * KPAD], DT16)
            idx_t = gk.tile([P, KT * KPAD], DTI16)
            nc.sync.dma_start(warr[:], warr_d[:])
            nc.sync.dma_start(idx_t[:], idx_d[:])

            a_tiles = [gk.tile([P, NPC], DT16, tag=f"A{kt}", name=f"A{kt}")
                       for kt in range(KT)]
            last_scatter = None
            for kt in range(KT):
                ls = nc.gpsimd.local_scatter(
                    a_tiles[kt][:],
                    warr[:, KPAD * kt:KPAD * (kt + 1)],
                    idx_t[:, KPAD * kt:KPAD * (kt + 1)],
                    channels=P, num_elems=NPC, num_idxs=KPAD,
                )
                add_dep_helper(ls.ins, lib.ins, reason="scatter after lib")
                last_scatter = ls

            xTf16 = gk.tile([P, (DIN // P) * N], DT16)
            wg16 = gk.tile([P, (DIN // P) * D], DT16)
            nc.sync.dma_start(xTf16[:], xTf_d[:])
            nc.sync.dma_start(wg16[:], wg_d[:])

            cols = keep.tile([P, 14], DT32)
            winT16 = keep.tile([P, 2 * 3 * D], DT16)
            ipb = keep.tile([P, 6], DT32)
            nc.sync.dma_start(cols[:], cols_d[:])
            nc.sync.dma_start(winT16[:], winT_d[:])
            nc.sync.dma_start(ipb[:], ipb_d[:])

            # ---- xw = x @ W_gcn (replicated, fp16) ----
            xws16f = gk.tile([P, KT * D], DT16)
            with tc.tile_pool(name="xw_ps", bufs=4, space="PSUM") as xps:
                for j in range(KT):
                    pxw = xps.tile([P, D], DT32, space="PSUM", tag="xw")
                    for k in range(DIN // P):
                        nc.tensor.matmul(
                            pxw[:],
                            lhsT=xTf16[:, N * k + P * j:N * k + P * (j + 1)],
                            rhs=wg16[:, D * k:D * (k + 1)],
                            start=(k == 0), stop=(k == DIN // P - 1))
                    nc.vector.tensor_copy(xws16f[:, D * j:D * (j + 1)], pxw[:])

            # ---- transposed aggregation: h^T[c*128+p, dst] ----
            hT16 = keep.tile([P, 2 * NPC], DT16)
            with tc.tile_pool(name="agg_ps", bufs=1, space="PSUM") as aps:
                hps = [aps.tile([P, NPC], DT32, space="PSUM",
                                tag=f"hps{c}", name=f"hps{c}")
                       for c in range(2)]
                for kt in range(KT):
                    for c in range(2):
                        mm = nc.tensor.matmul(
                            hps[c][:],
                            lhsT=xws16f[:, D * kt + P * c:D * kt + P * (c + 1)],
                            rhs=a_tiles[kt][:],
                            start=(kt == 0), stop=(kt == KT - 1))
                        if kt == 0 and c == 0:
                            add_dep_helper(mm.ins, last_scatter.ins,
                                           reason="agg after scatters")
                for c in range(2):
                    nc.scalar.activation(hT16[:, NPC * c:NPC * (c + 1)],
                                         hps[c][:], F.Relu,
                                         bias=cols[:, c:c + 1])

            ctx_gcn.__exit__(None, None, None)
            ak = ctx_attn = tc.tile_pool(name="attn_keep", bufs=1)
            ak = ctx_attn.__enter__()

            # ---- local q / K^T / V in fp8 (x16), one packed KV AllGather ----
            qT8 = ak.tile([P, H * NPC], DT8)
            kv8 = ak.tile([P, 4 * NPC], DT8)
            with tc.tile_pool(name="kv_ps", bufs=3, space="PSUM") as kvps:
                for hh in range(H):
                    pq = kvps.tile([P, NPC], DT32, space="PSUM", tag="kv")
                    for k in range(2):
                        nc.tensor.matmul(
                            pq[:],
                            lhsT=winT16[:, 768 * k + P * hh:768 * k + P * (hh + 1)],
                            rhs=hT16[:, NPC * k:NPC * (k + 1)],
                            start=(k == 0), stop=(k == 1))
                    nc.vector.tensor_scalar(
                        qT8[:, NPC * hh:NPC * (hh + 1)], pq[:],
                        ipb[:, hh:hh + 1], FSC, op0=A.add, op1=A.mult)
                    pk = kvps.tile([P, NPC], DT32, space="PSUM", tag="kv")
                    for k in range(2):
                        nc.tensor.matmul(
                            pk[:],
                            lhsT=winT16[:, 768 * k + D + P * hh:
                                        768 * k + D + P * (hh + 1)],
                            rhs=hT16[:, NPC * k:NPC * (k + 1)],
                            start=(k == 0), stop=(k == 1))
                    nc.vector.tensor_scalar(
                        kv8[:, NPC * hh:NPC * (hh + 1)], pk[:],
                        ipb[:, 2 + hh:3 + hh], FSC, op0=A.add, op1=A.mult)
                    for m in range(MPC):
                        pv = kvps.tile([P, P], DT32, space="PSUM", tag="kvv")
                        for k in range(2):
                            nc.tensor.matmul(
                                pv[:],
                                lhsT=hT16[:, NPC * k + P * m:NPC * k + P * (m + 1)],
                                rhs=winT16[:, 768 * k + 2 * D + P * hh:
                                            768 * k + 2 * D + P * (hh + 1)],
                                start=(k == 0), stop=(k == 1))
                        nc.vector.tensor_scalar(
                            kv8[:, NPC * (2 + hh) + P * m:
                                NPC * (2 + hh) + P * (m + 1)], pv[:],
                            FSC, None, op0=A.mult)

            kv_bounce = dram.tile([4 * P, NPC], DT8)
            kv_gath = dram.tile([N_CORES * 4 * P, NPC], DT8,
                                addr_space="Shared")
            nc.scalar.dma_start(
                kv_bounce[:].rearrange("(x p) n -> p x n", p=P),
                kv8[:].rearrange("p (x n) -> p x n", x=4))
            nc.gpsimd.collective_compute(
                "AllGather", A.bypass,
                replica_groups=[list(range(N_CORES))],
                ins=[kv_bounce.opt()], outs=[kv_gath.opt()])

            # FFN / out-proj weights stream while the AllGather runs
            w1T16 = ak.tile([P, 2 * DFF], DT16)
            nc.sync.dma_start(w1T16[:], w1T_d[:])
            w2T216 = ak.tile([P, (DFF // P) * D], DT16)
            nc.sync.dma_start(w2T216[:], w2T2_d[:])
            woT216 = ak.tile([P, 4 * P], DT16)
            nc.sync.dma_start(woT216[:], woT2_d[:])
            b1t = ak.tile([P, DFF // P], DT32)
            nc.sync.dma_start(b1t[:], b1_d[:])

            # ---- chunked loads of gathered K^T / V (per source rank) ----
            kT8 = ak.tile([P, H * N], DT8)
            v8 = ak.tile([P, H * N], DT8)
            gv = kv_gath[:].rearrange("(g x p) n -> g x p n",
                                      g=N_CORES, x=4, p=P)
            for g in range(N_CORES):
                for hh in range(H):
                    nc.scalar.dma_start(
                        kT8[:, N * hh + NPC * g:N * hh + NPC * (g + 1)],
                        gv[g, hh])
                for hh in range(H):
                    nc.scalar.dma_start(
                        v8[:, N * hh + NPC * g:N * hh + NPC * (g + 1)],
                        gv[g, 2 + hh])

            # ---- S^T -> exp(fp8) -> DoubleRow PV + DoubleRow denominators ----
            oS16 = ak.tile([P, H * NPC], DT16)
            rec16 = ak.tile([1, H * NPC], DT16)
            with tc.tile_pool(name="att_sb", bufs=3) as atsb, \
                 tc.tile_pool(name="att_ps", bufs=1, space="PSUM") as atps:
                o_ps = [atps.tile([P, NPC], DT32, space="PSUM",
                                  tag=f"o{hh}", name=f"o{hh}")
                        for hh in range(H)]
                sum_ps = [atps.tile([1, NPC], DT32, space="PSUM",
                                    tag=f"sm{hh}", name=f"sm{hh}")
                          for hh in range(H)]
                with tc.tile_pool(name="s_ps", bufs=2, space="PSUM") as sps:
                    for kt2 in range(KT // 2):
                        for hh in range(H):
                            ps_s = sps.tile([P, 2 * NPC], DT32, space="PSUM",
                                            tag="S")
                            for u in range(2):
                                kt = 2 * kt2 + u
                                nc.tensor.matmul(
                                    ps_s[:, NPC * u:NPC * (u + 1)],
                                    lhsT=kT8[:, N * hh + P * kt:
                                             N * hh + P * (kt + 1)],
                                    rhs=qT8[:, NPC * hh:NPC * (hh + 1)],
                                    start=True, stop=True)
                            es = atsb.tile([P, 2 * NPC], DT8, tag="es")
                            nc.scalar.activation(es[:], ps_s[:], F.Exp,
                                                 scale=EXP_SCALE)
                            es2 = es[:].rearrange("p (two n) -> p two n",
                                                  two=2)
                            nc.tensor.matmul(
                                o_ps[hh][:],
                                lhsT=v8[:, N * hh + 2 * P * kt2:
                                        N * hh + 2 * P * (kt2 + 1)].rearrange(
                                    "p (two f) -> p two f", two=2),
                                rhs=es2, perf_mode=DR,
                                start=(kt2 == 0), stop=(kt2 == KT // 2 - 1))
                            nc.tensor.matmul(
                                sum_ps[hh][:],
                                lhsT=ones8[:].rearrange(
                                    "p (two f) -> p two f", two=2)[:, :, 0:1],
                                rhs=es2, perf_mode=DR,
                                start=(kt2 == 0), stop=(kt2 == KT // 2 - 1))

                # reciprocal rows -> broadcast -> scale o
                with tc.tile_pool(name="rb_ps", bufs=2, space="PSUM") as rps:
                    for hh in range(H):
                        with nc.allow_low_precision(
                                reason="softmax denom ~4096, f16 rel ok"):
                            nc.vector.reciprocal(
                                rec16[:, NPC * hh:NPC * (hh + 1)],
                                sum_ps[hh][:])
                        rbc = rps.tile([P, NPC], DT32, space="PSUM", tag="rbc")
                        nc.tensor.matmul(
                            rbc[:], lhsT=ones16r[:],
                            rhs=rec16[:, NPC * hh:NPC * (hh + 1)],
                            start=True, stop=True)
                        rb16 = atsb.tile([P, NPC], DT16, tag="rb16")
                        nc.vector.tensor_copy(rb16[:], rbc[:])
                        nc.vector.tensor_tensor(
                            oS16[:, NPC * hh:NPC * (hh + 1)],
                            o_ps[hh][:], rb16[:], op=A.mult)

            # ---- out_proj^T + residual + LN1^T (all feature-major) ----
            h1T16 = ak.tile([P, 2 * NPC], DT16)
            x1h = ak.tile([P, 2 * NPC], DT16)
            with tc.tile_pool(name="ln_sb", bufs=2) as lsb:
                with tc.tile_pool(name="op_ps", bufs=1, space="PSUM") as ops:
                    x1_ps = [ops.tile([P, NPC], DT32, space="PSUM",
                                      tag=f"x1{c}", name=f"x1{c}")
                             for c in range(2)]
                    for c in range(2):
                        for hh in range(H):
                            nc.tensor.matmul(
                                x1_ps[c][:],
                                lhsT=woT216[:, P * (2 * hh + c):
                                            P * (2 * hh + c + 1)],
                                rhs=oS16[:, NPC * hh:NPC * (hh + 1)],
                                start=(hh == 0), stop=(hh == 1))
                    for c in range(2):
                        nc.vector.scalar_tensor_tensor(
                            x1h[:, NPC * c:NPC * (c + 1)], x1_ps[c][:],
                            cols[:, 2 + c:3 + c],
                            hT16[:, NPC * c:NPC * (c + 1)],
                            op0=A.add, op1=A.add)

                def layernorm_T(dst, xh, gcol, bcol, out_dt, tag):
                    """LN over features (partition dim x 2 chunks), rows via
                    ones-matmuls. xh: [P, 2*NPC] f16. dst written per chunk."""
                    with tc.tile_pool(name=f"ln_ps_{tag}", bufs=1,
                                      space="PSUM") as rws:
                        mu_ps = rws.tile([1, NPC], DT32, space="PSUM",
                                         tag=f"{tag}mu")
                        msq_ps = rws.tile([1, NPC], DT32, space="PSUM",
                                          tag=f"{tag}ms")
                        sq = lsb.tile([P, 2 * NPC], DT16, tag=f"{tag}sq")
                        nc.vector.tensor_tensor(sq[:], xh[:], xh[:], op=A.mult)
                        for c in range(2):
                            nc.tensor.matmul(
                                mu_ps[:], lhsT=ones16c[:],
                                rhs=xh[:, NPC * c:NPC * (c + 1)],
                                start=(c == 0), stop=(c == 1))
                        for c in range(2):
                            nc.tensor.matmul(
                                msq_ps[:], lhsT=ones16c[:],
                                rhs=sq[:, NPC * c:NPC * (c + 1)],
                                start=(c == 0), stop=(c == 1))
                        mu_n = lsb.tile([1, NPC], DT32, tag=f"{tag}mn")
                        nc.vector.tensor_scalar(mu_n[:], mu_ps[:], 1.0 / D,
                                                None, op0=A.mult)
                        nmu16 = lsb.tile([1, NPC], DT16, tag=f"{tag}nm")
                        nc.vector.tensor_scalar(nmu16[:], mu_ps[:], -1.0 / D,
                                                None, op0=A.mult)
                        mu2 = lsb.tile([1, NPC], DT32, tag=f"{tag}m2")
                        nc.vector.tensor_tensor(mu2[:], mu_n[:], mu_n[:],
                                                op=A.mult)
                        var = lsb.tile([1, NPC], DT32, tag=f"{tag}vr")
                        nc.vector.scalar_tensor_tensor(
                            var[:], msq_ps[:], 1.0 / D, mu2[:],
                            op0=A.mult, op1=A.subtract)
                        sd = lsb.tile([1, NPC], DT32, tag=f"{tag}sd")
                        nc.scalar.activation(sd[:], var[:], F.Sqrt,
                                             bias=eps1[:])
                        rstd16 = lsb.tile([1, NPC], DT16, tag=f"{tag}rs")
                        with nc.allow_low_precision(
                                reason="rstd row f16, rel 1e-3 ok"):
                            nc.vector.reciprocal(rstd16[:], sd[:])
                        nmu_bc = rws.tile([P, NPC], DT32, space="PSUM",
                                          tag=f"{tag}nb")
                        rstd_bc = rws.tile([P, NPC], DT32, space="PSUM",
                                           tag=f"{tag}rb")
                        nc.tensor.matmul(nmu_bc[:], lhsT=ones16r[:],
                                         rhs=nmu16[:], start=True, stop=True)
                        nc.tensor.matmul(rstd_bc[:], lhsT=ones16r[:],
                                         rhs=rstd16[:], start=True, stop=True)
                        for c in range(2):
                            t = lsb.tile([P, NPC],
                                         DT16 if out_dt == DT16 else DT32,
                                         tag=f"{tag}t")
                            nc.vector.tensor_tensor(
                                t[:], xh[:, NPC * c:NPC * (c + 1)], nmu_bc[:],
                                op=A.add)
                            t2 = lsb.tile([P, NPC],
                                          DT16 if out_dt == DT16 else DT32,
                                          tag=f"{tag}t2")
                            nc.vector.tensor_tensor(t2[:], t[:], rstd_bc[:],
                                                    op=A.mult)
                            nc.vector.tensor_scalar(
                                dst[:, NPC * c:NPC * (c + 1)], t2[:],
                                gcol[:, c:c + 1], bcol[:, c:c + 1],
                                op0=A.mult, op1=A.add)

                layernorm_T(h1T16, x1h, cols[:, 6:8], cols[:, 8:10], DT16, "a")

                # ---- FFN (fp16, transposed ff2) ----
                ff1T = ak.tile([P, (DFF // P) * NPC], DT16)
                with tc.tile_pool(name="f1_ps", bufs=3, space="PSUM") as fps:
                    for dc in range(DFF // P):
                        pf = fps.tile([P, NPC], DT32, space="PSUM", tag="f1")
                        for k in range(2):
                            nc.tensor.matmul(
                                pf[:],
                                lhsT=w1T16[:, DFF * k + P * dc:
                                           DFF * k + P * (dc + 1)],
                                rhs=h1T16[:, NPC * k:NPC * (k + 1)],
                                start=(k == 0), stop=(k == 1))
                        nc.scalar.activation(
                            ff1T[:, NPC * dc:NPC * (dc + 1)], pf[:], F.Relu,
                            bias=b1t[:, dc:dc + 1])

                x2h = lsb.tile([P, 2 * NPC], DT16, tag="x2h")
                with tc.tile_pool(name="f2_ps", bufs=1, space="PSUM") as fps2:
                    x2_ps = [fps2.tile([P, NPC], DT32, space="PSUM",
                                       tag=f"x2{c}", name=f"x2{c}")
                             for c in range(2)]
                    for dc in range(DFF // P):
                        for c in range(2):
                            nc.tensor.matmul(
                                x2_ps[c][:],
                                lhsT=w2T216[:, P * (2 * dc + c):
                                            P * (2 * dc + c + 1)],
                                rhs=ff1T[:, NPC * dc:NPC * (dc + 1)],
                                start=(dc == 0), stop=(dc == DFF // P - 1))
                    for c in range(2):
                        nc.vector.scalar_tensor_tensor(
                            x2h[:, NPC * c:NPC * (c + 1)], x2_ps[c][:],
                            cols[:, 4 + c:5 + c],
                            h1T16[:, NPC * c:NPC * (c + 1)],
                            op0=A.add, op1=A.add)

                out_sb = ak.tile([P, 2 * NPC], DT32)
                layernorm_T(out_sb, x2h, cols[:, 10:12], cols[:, 12:14],
                            DT32, "b")
                nc.scalar.dma_start(out_d[:], out_sb[:])
            ctx_attn.__exit__(None, None, None)

    nc.compile()
    return nc


# ======================= host-side prep =======================

def _prep_inputs(x, edge_index, edge_weight, W_gcn, b_gcn, in_proj_w,
                 in_proj_b, out_proj_w, out_proj_b, lin1_w, lin1_b, lin2_w,
                 lin2_b, ln1_g, ln1_b, ln2_g, ln2_b):
    """Index-permutation / layout prep + edge-weight prenormalization."""
    x = np.asarray(x, np.float32)
    src = np.asarray(edge_index[0], np.int64)
    dst = np.asarray(edge_index[1], np.int64)
    w = np.asarray(edge_weight, np.float64)

    def wrap128(a):
        n = a.shape[0] // P
        return np.ascontiguousarray(
            a.reshape(n, P, a.shape[1]).transpose(1, 0, 2).reshape(P, -1))

    def colsof(v):
        return np.ascontiguousarray(
            np.asarray(v, np.float32).reshape(2, P).T)

    f16 = np.float16
    deg = np.zeros(N, np.float64)
    np.add.at(deg, dst, w)
    deg += 1.0
    dinv = 1.0 / np.sqrt(deg)
    norm = (dinv[src] * w * dinv[dst]).astype(np.float32)

    ipb_np = np.asarray(in_proj_b, np.float32)
    bv = ipb_np[2 * D:]
    bo_eff = (np.asarray(out_proj_w, np.float32) @ bv
              + np.asarray(out_proj_b, np.float32))

    wo = np.asarray(out_proj_w, np.float32)
    woT2 = np.empty((P, 4 * P), np.float32)
    for hh in range(H):
        for c in range(2):
            # lhsT[p, m] = Wo[c*128+m, hh*128+p] / FSC
            woT2[:, P * (2 * hh + c):P * (2 * hh + c + 1)] = \
                wo[c * P:(c + 1) * P, hh * P:(hh + 1) * P].T / FSC

    w2 = np.asarray(lin2_w, np.float32)
    w2T2 = np.empty((P, (DFF // P) * D), np.float32)
    for dc in range(DFF // P):
        for c in range(2):
            w2T2[:, P * (2 * dc + c):P * (2 * dc + c + 1)] = \
                w2[c * P:(c + 1) * P, dc * P:(dc + 1) * P].T

    cols = np.concatenate([
        colsof(b_gcn), colsof(bo_eff), colsof(lin2_b),
        colsof(ln1_g), colsof(ln1_b), colsof(ln2_g), colsof(ln2_b)], axis=1)

    shared = {
        "xTf": wrap128(np.ascontiguousarray(x.T)).astype(f16),
        "wg": wrap128(np.asarray(W_gcn, np.float32)).astype(f16),
        "winT": wrap128(np.ascontiguousarray(
            np.asarray(in_proj_w, np.float32).T)).astype(f16),
        "ipb": np.ascontiguousarray(ipb_np.reshape(6, P).T),
        "woT2": woT2.astype(f16),
        "w1T": wrap128(np.ascontiguousarray(
            np.asarray(lin1_w, np.float32).T)).astype(f16),
        "b1": np.ascontiguousarray(
            np.asarray(lin1_b, np.float32).reshape(DFF // P, P).T),
        "w2T2": w2T2.astype(f16),
        "cols": cols,
    }

    core_of = dst // NPC
    in_maps = []
    for c in range(N_CORES):
        sel = np.nonzero(core_of == c)[0]
        s_c = src[sel]
        d_c = (dst[sel] - NPC * c).astype(np.int64)
        n_c = norm[sel]

        w_arr = np.zeros((N, KPAD), np.float32)
        idx_arr = np.full((N, KPAD), -1, np.int16)
        counts = np.zeros(N, np.int32)
        slot_of = {}
        for si, di, wi in zip(s_c.tolist(), d_c.tolist(), n_c.tolist()):
            key = si * NPC + di
            slot = slot_of.get(key)
            if slot is None:
                j = int(counts[si])
                assert j < KPAD, f"KPAD overflow at src {si}"
                counts[si] = j + 1
                w_arr[si, j] = wi
                idx_arr[si, j] = di
                slot_of[key] = j
            else:
                w_arr[si, slot] += wi
        # self loops: weight dinv^2 at (g, g-512c)
        for di in range(NPC):
            g = NPC * c + di
            key = g * NPC + di
            slot = slot_of.get(key)
            if slot is None:
                j = int(counts[g])
                assert j < KPAD, f"KPAD overflow at self {g}"
                counts[g] = j + 1
                w_arr[g, j] = dinv[g] * dinv[g]
                idx_arr[g, j] = di
            else:
                w_arr[g, slot] += dinv[g] * dinv[g]

        in_maps.append({
            **shared,
            "warr": wrap128(w_arr).astype(f16),
            "idx": wrap128(idx_arr),
        })
    return in_maps


# ======================= runner =======================

class _Runner:
    """Persistent-jit SPMD executor (mirrors bass2jax.run_bass_via_pjrt)."""

    def __init__(self, nc):
        import jax
        from jax.sharding import Mesh, PartitionSpec
        from jax.experimental.shard_map import shard_map
        from concourse.bass2jax import (_bass_exec_p, install_neuronx_cc_hook,
                                        partition_id_tensor)
        install_neuronx_cc_hook()
        self.jax = jax
        partition_name = (nc.partition_id_tensor.name
                          if nc.partition_id_tensor else None)
        in_names, out_names, out_avals, zero_outs = [], [], [], []
        for alloc in nc.m.functions[0].allocations:
            if not isinstance(alloc, mybir.MemoryLocationSet):
                continue
            name = alloc.memorylocations[0].name
            if alloc.kind == "ExternalInput":
                if name != partition_name:
                    in_names.append(name)
            elif alloc.kind == "ExternalOutput":
                out_names.append(name)
                shape = tuple(alloc.tensor_shape)
                dtype = mybir.dt.np(alloc.dtype)
                out_avals.append(jax.core.ShapedArray(shape, dtype))
                zero_outs.append(np.zeros(shape, dtype))
        self.in_names, self.out_names = in_names, out_names
        self.out_shapes = [tuple(a.shape) for a in out_avals]
        self.n_params = len(in_names)
        self.zero_outs = zero_outs
        all_in = in_names + out_names
        if partition_name is not None:
            all_in.append(partition_name)

        def _body(*args):
            operands = list(args)
            if partition_name is not None:
                operands.append(partition_id_tensor())
            return tuple(_bass_exec_p.bind(
                *operands, out_avals=tuple(out_avals), in_names=tuple(all_in),
                out_names=tuple(out_names), lowering_input_output_aliases=(),
                sim_require_finite=True, sim_require_nnan=True, nc=nc))

        devices = jax.devices()[:N_CORES]
        self.mesh = Mesh(np.asarray(devices), ("core",))
        nin = self.n_params + len(out_names)
        self.fn = jax.jit(
            shard_map(_body, mesh=self.mesh,
                      in_specs=(PartitionSpec("core"),) * nin,
                      out_specs=(PartitionSpec("core"),) * len(out_names),
                      check_rep=False),
            keep_unused=True)

    def place(self, in_maps):
        import jax
        from jax.sharding import PartitionSpec
        per_core = [[np.asarray(m[n]) for n in self.in_names] for m in in_maps]
        concat = [np.concatenate([per_core[c][i] for c in range(N_CORES)], axis=0)
                  for i in range(self.n_params)]
        zeros = [np.zeros((N_CORES * z.shape[0], *z.shape[1:]), z.dtype)
                 for z in self.zero_outs]
        sh = jax.sharding.NamedSharding(self.mesh, PartitionSpec("core"))
        return [jax.device_put(a, sh) for a in (*concat, *zeros)]

    def run(self, args):
        outs = self.fn(*args)
        self.jax.block_until_ready(outs)
        return outs

    def results(self, outs):
        res = []
        for c in range(N_CORES):
            d = {}
            for i, name in enumerate(self.out_names):
                full = np.asarray(outs[i])
                ps = self.out_shapes[i]
                d[name] = full.reshape((N_CORES,) + ps)[c]
            res.append(d)
        return res


_CACHE = {}


def _get_runner():
    if "runner" not in _CACHE:
        nc = build_kernel()
        _CACHE["nc"] = nc
        _CACHE["runner"] = _Runner(nc)
    return _CACHE["runner"]


def kernel(**inputs) -> np.ndarray:
    runner = _get_runner()
    in_maps = _prep_inputs(**inputs)
    args = runner.place(in_maps)
    outs = runner.run(args)
    res = runner.results(outs)
    # out_d is out^T wrapped: [P, 2, NPC]; out[n, c*128+p] = arr[p, c, n]
    full = np.empty((N, D), np.float32)
    for c in range(N_CORES):
        arr = res[c]["out"].reshape(P, 2, NPC)
        full[NPC * c:NPC * (c + 1)] = arr.transpose(2, 1, 0).reshape(NPC, D)
    return full


# revision 24
# speedup vs baseline: 1.2279x; 1.0793x over previous
"""GCNEncoder (GCNConv + TransformerEncoderLayer) on 8 Trainium2 NeuronCores.

Sharding: nodes split 512/core. Per core:
  - GCN: dense normalized-adjacency blocks A [4096 src, 512 dst] built on
    device via GPSIMD local_scatter from host-prenormalized edge values
    (deg/dinv/dup-merge/self-loops folded in at host); aggregation runs
    TRANSPOSED (h^T = xw^T-stationary @ A) so h lands feature-major with no
    transposes anywhere in the pipeline.
  - Attention in fp8: q/K^T/V cast to fp8e4 (x16 scaled), ONE 2MB fp8
    AllGather of K/V, per-rank chunked gather loads so scores start early.
    exp probs kept fp8; PV and the softmax denominators use fp8 DoubleRow
    matmuls (2x, contraction pairs are the natural tile layout).
  - Post-attention stays transposed: o scaled by reciprocal-row broadcast,
    out_proj^T, LayerNorms via ones-matmul row reductions, FFN fp16 with
    ff2^T, final LN2^T written transposed; host un-transposes the output.
"""

import math

import numpy as np

import concourse.bacc as bacc
import concourse.mybir as mybir
import concourse.tile as tile
from concourse import library_config
from concourse.tile_rust import add_dep_helper

N_CORES = 8
N = 4096
E = 131072
DIN = 512
D = 256
H = 2
DH = 128
DFF = 2048
EPS = 1e-5
P = 128

NPC = N // N_CORES          # nodes per core = 512
MPC = NPC // P              # m-chunks per core = 4
KT = N // P                 # src k-tiles = 32
KPAD = 32                   # max out-edges per (core, src-node)
FSC = 16.0                  # fp8 pre-scale for q/k/v
DT8 = mybir.dt.float8e4
DT16 = mybir.dt.float16
DT32 = mybir.dt.float32
DTI16 = mybir.dt.int16
F = mybir.ActivationFunctionType
A = mybir.AluOpType
DR = mybir.MatmulPerfMode.DoubleRow
INV_SQRT_DH = 1.0 / math.sqrt(DH)
EXP_SCALE = INV_SQRT_DH / (FSC * FSC)


def build_kernel():
    nc = bacc.Bacc("TRN2", target_bir_lowering=False, debug=False,
                   num_devices=N_CORES)

    def din(name, shape, dt=DT32):
        return nc.dram_tensor(name, shape, dt, kind="ExternalInput")

    warr_d = din("warr", [P, KT * KPAD], DT16)
    idx_d = din("idx", [P, KT * KPAD], DTI16)
    xTf_d = din("xTf", [P, (DIN // P) * N], DT16)   # full x.T wrapped
    wg_d = din("wg", [P, (DIN // P) * D], DT16)
    winT_d = din("winT", [P, 2 * 3 * D], DT16)
    ipb_d = din("ipb", [P, 6])
    woT2_d = din("woT2", [P, 4 * P], DT16)
    w1T_d = din("w1T", [P, 2 * DFF], DT16)
    b1_d = din("b1", [P, DFF // P])
    w2T2_d = din("w2T2", [P, (DFF // P) * D], DT16)
    cols_d = din("cols", [P, 14])
    ident_d = din("ident", [P, P], DT16)

    out_d = nc.dram_tensor("out", [P, 2 * NPC], DT32, kind="ExternalOutput")

    with tile.TileContext(nc) as tc:
        with (
            tc.tile_pool(name="keep", bufs=1) as keep,
            tc.tile_pool(name="dram", bufs=1, space="DRAM") as dram,
        ):
            ones8 = keep.tile([P, 32], DT8)
            ones16c = keep.tile([P, 1], DT16)
            ones16r = keep.tile([1, P], DT16)
            eps128 = keep.tile([P, 1], DT32)
            nc.vector.memset(ones8[:], 1.0)
            nc.vector.memset(ones16c[:], 1.0)
            nc.vector.memset(ones16r[:], 1.0)
            nc.vector.memset(eps128[:], EPS)
            ident16 = keep.tile([P, P], DT16)
            nc.sync.dma_start(ident16[:], ident_d[:])
            ident1 = keep.tile([1, 1], DT32)
            nc.vector.memset(ident1[:], 1.0)

            lib = nc.gpsimd.load_library(library_config.local_scatter)

            gk = ctx_gcn = tc.tile_pool(name="gcn_keep", bufs=1)
            gk = ctx_gcn.__enter__()

            # ---- A-build inputs first: scatters on GpSimd start ASAP ----
            warr = gk.tile([P, KT * KPAD], DT16)
            idx_t = gk.tile([P, KT * KPAD], DTI16)
            nc.sync.dma_start(warr[:], warr_d[:])
            nc.sync.dma_start(idx_t[:], idx_d[:])

            a_tiles = [gk.tile([P, NPC], DT16, tag=f"A{kt}", name=f"A{kt}")
                       for kt in range(KT)]
            last_scatter = None
            for kt in range(KT):
                ls = nc.gpsimd.local_scatter(
                    a_tiles[kt][:],
                    warr[:, KPAD * kt:KPAD * (kt + 1)],
                    idx_t[:, KPAD * kt:KPAD * (kt + 1)],
                    channels=P, num_elems=NPC, num_idxs=KPAD,
                )
                add_dep_helper(ls.ins, lib.ins, reason="scatter after lib")
                last_scatter = ls

            xTf16 = gk.tile([P, (DIN // P) * N], DT16)
            wg16 = gk.tile([P, (DIN // P) * D], DT16)
            nc.sync.dma_start(xTf16[:], xTf_d[:])
            nc.sync.dma_start(wg16[:], wg_d[:])

            cols = keep.tile([P, 14], DT32)
            winT16 = keep.tile([P, 2 * 3 * D], DT16)
            ipb = keep.tile([P, 6], DT32)
            nc.sync.dma_start(cols[:], cols_d[:])
            nc.sync.dma_start(winT16[:], winT_d[:])
            nc.sync.dma_start(ipb[:], ipb_d[:])

            # ---- xw = x @ W_gcn (replicated, fp16) ----
            xws16f = gk.tile([P, KT * D], DT16)
            with tc.tile_pool(name="xw_ps", bufs=4, space="PSUM") as xps:
                for j in range(KT):
                    pxw = xps.tile([P, D], DT32, space="PSUM", tag="xw")
                    for k in range(DIN // P):
                        nc.tensor.matmul(
                            pxw[:],
                            lhsT=xTf16[:, N * k + P * j:N * k + P * (j + 1)],
                            rhs=wg16[:, D * k:D * (k + 1)],
                            start=(k == 0), stop=(k == DIN // P - 1))
                    nc.vector.tensor_copy(xws16f[:, D * j:D * (j + 1)], pxw[:])

            # ---- transposed aggregation: h^T[c*128+p, dst] ----
            hT16 = keep.tile([P, 2 * NPC], DT16)
            with tc.tile_pool(name="agg_ps", bufs=1, space="PSUM") as aps:
                hps = [aps.tile([P, NPC], DT32, space="PSUM",
                                tag=f"hps{c}", name=f"hps{c}")
                       for c in range(2)]
                for kt in range(KT):
                    for c in range(2):
                        mm = nc.tensor.matmul(
                            hps[c][:],
                            lhsT=xws16f[:, D * kt + P * c:D * kt + P * (c + 1)],
                            rhs=a_tiles[kt][:],
                            start=(kt == 0), stop=(kt == KT - 1))
                        if kt == 0 and c == 0:
                            add_dep_helper(mm.ins, last_scatter.ins,
                                           reason="agg after scatters")
                for c in range(2):
                    nc.scalar.activation(hT16[:, NPC * c:NPC * (c + 1)],
                                         hps[c][:], F.Relu,
                                         bias=cols[:, c:c + 1])

            ctx_gcn.__exit__(None, None, None)
            ak = ctx_attn = tc.tile_pool(name="attn_keep", bufs=1)
            ak = ctx_attn.__enter__()

            # ---- local K^T / V in fp8 (x16), one packed KV AllGather; q
            #      computed while the collective runs ----
            qT8 = ak.tile([P, H * NPC], DT8)
            kv8 = ak.tile([P, 4 * NPC], DT8)
            with tc.tile_pool(name="kv_ps", bufs=3, space="PSUM") as kvps:
                for hh in range(H):
                    pk = kvps.tile([P, NPC], DT32, space="PSUM", tag="kv")
                    for k in range(2):
                        nc.tensor.matmul(
                            pk[:],
                            lhsT=winT16[:, 768 * k + D + P * hh:
                                        768 * k + D + P * (hh + 1)],
                            rhs=hT16[:, NPC * k:NPC * (k + 1)],
                            start=(k == 0), stop=(k == 1))
                    nc.vector.tensor_scalar(
                        kv8[:, NPC * hh:NPC * (hh + 1)], pk[:],
                        ipb[:, 2 + hh:3 + hh], FSC, op0=A.add, op1=A.mult)
                    for m in range(MPC):
                        pv = kvps.tile([P, P], DT32, space="PSUM", tag="kvv")
                        for k in range(2):
                            nc.tensor.matmul(
                                pv[:],
                                lhsT=hT16[:, NPC * k + P * m:NPC * k + P * (m + 1)],
                                rhs=winT16[:, 768 * k + 2 * D + P * hh:
                                            768 * k + 2 * D + P * (hh + 1)],
                                start=(k == 0), stop=(k == 1))
                        nc.vector.tensor_scalar(
                            kv8[:, NPC * (2 + hh) + P * m:
                                NPC * (2 + hh) + P * (m + 1)], pv[:],
                            FSC, None, op0=A.mult)

                # bounce is a plain partition-line copy: 2KB descriptors
                kv_bounce = dram.tile([P, 4 * NPC], DT8)
                kv_gath = dram.tile([N_CORES * P, 4 * NPC], DT8,
                                    addr_space="Shared")
                nc.scalar.dma_start(kv_bounce[:], kv8[:])
                nc.gpsimd.collective_compute(
                    "AllGather", A.bypass,
                    replica_groups=[list(range(N_CORES))],
                    ins=[kv_bounce.opt()], outs=[kv_gath.opt()])

                for hh in range(H):
                    pq = kvps.tile([P, NPC], DT32, space="PSUM", tag="kv")
                    for k in range(2):
                        nc.tensor.matmul(
                            pq[:],
                            lhsT=winT16[:, 768 * k + P * hh:768 * k + P * (hh + 1)],
                            rhs=hT16[:, NPC * k:NPC * (k + 1)],
                            start=(k == 0), stop=(k == 1))
                    nc.vector.tensor_scalar(
                        qT8[:, NPC * hh:NPC * (hh + 1)], pq[:],
                        ipb[:, hh:hh + 1], FSC, op0=A.add, op1=A.mult)

            # FFN / out-proj weights stream while the AllGather runs
            w1T16 = ak.tile([P, 2 * DFF], DT16)
            nc.sync.dma_start(w1T16[:], w1T_d[:])
            w2T216 = ak.tile([P, (DFF // P) * D], DT16)
            nc.sync.dma_start(w2T216[:], w2T2_d[:])
            woT216 = ak.tile([P, 4 * P], DT16)
            nc.sync.dma_start(woT216[:], woT2_d[:])
            b1t = ak.tile([P, DFF // P], DT32)
            nc.sync.dma_start(b1t[:], b1_d[:])

            # ---- per-rank loads of gathered K/V: 2KB contiguous lines ----
            # kv_all[:, 2048*g + x*512 + n]: rank g's kv8 partition lines
            # (x: 0/1 = K^T h0/h1 per dh-partition, 2/3 = V h0/h1 node-major)
            kv_all = ak.tile([P, N_CORES * 4 * NPC], DT8)
            for g in range(N_CORES):
                nc.scalar.dma_start(
                    kv_all[:, 4 * NPC * g:4 * NPC * (g + 1)],
                    kv_gath[P * g:P * (g + 1), :])

            def kslice(hh, kt):
                g, ktl = kt // MPC, kt % MPC
                base = 4 * NPC * g + NPC * hh + P * ktl
                return kv_all[:, base:base + P]

            def vpair(hh, kt2):
                g, ml = kt2 // 2, kt2 % 2
                base = 4 * NPC * g + NPC * (2 + hh) + 2 * P * ml
                return kv_all[:, base:base + 2 * P].rearrange(
                    "p (two f) -> p two f", two=2)

            # ---- S^T -> exp(fp8) -> DoubleRow PV + DoubleRow denominators ----
            oS16 = ak.tile([P, H * NPC], DT16)
            with tc.tile_pool(name="att_sb", bufs=3) as atsb, \
                 tc.tile_pool(name="att_ps", bufs=1, space="PSUM") as atps:
                o_ps = [atps.tile([P, NPC], DT32, space="PSUM",
                                  tag=f"o{hh}", name=f"o{hh}")
                        for hh in range(H)]
                sum_ps = [atps.tile([1, NPC], DT32, space="PSUM",
                                    tag=f"sm{hh}", name=f"sm{hh}")
                          for hh in range(H)]
                with tc.tile_pool(name="s_ps", bufs=2, space="PSUM") as sps:
                    for kt2 in range(KT // 2):
                        for hh in range(H):
                            ps_s = sps.tile([P, 2 * NPC], DT32, space="PSUM",
                                            tag="S")
                            for u in range(2):
                                kt = 2 * kt2 + u
                                nc.tensor.matmul(
                                    ps_s[:, NPC * u:NPC * (u + 1)],
                                    lhsT=kslice(hh, kt),
                                    rhs=qT8[:, NPC * hh:NPC * (hh + 1)],
                                    start=True, stop=True)
                            es = atsb.tile([P, 2 * NPC], DT8, tag="es")
                            nc.scalar.activation(es[:], ps_s[:], F.Exp,
                                                 scale=EXP_SCALE)
                            es2 = es[:].rearrange("p (two n) -> p two n",
                                                  two=2)
                            nc.tensor.matmul(
                                o_ps[hh][:],
                                lhsT=vpair(hh, kt2),
                                rhs=es2, perf_mode=DR,
                                start=(kt2 == 0), stop=(kt2 == KT // 2 - 1))
                            nc.tensor.matmul(
                                sum_ps[hh][:],
                                lhsT=ones8[:].rearrange(
                                    "p (two f) -> p two f", two=2)[:, :, 0:1],
                                rhs=es2, perf_mode=DR,
                                start=(kt2 == 0), stop=(kt2 == KT // 2 - 1))

                # denominators: pack rows into partitions (reciprocal free
                # size is the cost driver), recip [128,8], broadcast back
                # via identity matmuls
                with tc.tile_pool(name="rb_ps", bufs=2, space="PSUM") as rps:
                    srow32 = atsb.tile([1, H * NPC], DT32, tag="srow")
                    for hh in range(H):
                        nc.vector.tensor_copy(
                            srow32[:, NPC * hh:NPC * (hh + 1)], sum_ps[hh][:])
                    packT = rps.tile([P, 2 * MPC], DT32, space="PSUM",
                                     tag="packT")
                    for j in range(2 * MPC):
                        nc.tensor.transpose(
                            packT[:, j:j + 1], srow32[:, P * j:P * (j + 1)],
                            ident1[:])
                    recT16 = atsb.tile([P, 2 * MPC], DT16, tag="recT")
                    with nc.allow_low_precision(
                            reason="softmax denom ~4096, f16 rel ok"):
                        nc.vector.reciprocal(recT16[:], packT[:])
                    for hh in range(H):
                        rbc = rps.tile([P, NPC], DT32, space="PSUM", tag="rbc")
                        for m in range(MPC):
                            nc.tensor.matmul(
                                rbc[:, P * m:P * (m + 1)],
                                lhsT=recT16[:, MPC * hh + m:
                                            MPC * hh + m + 1].to_broadcast(
                                    [P, P]),
                                rhs=ident16[:], start=True, stop=True)
                        rb16 = atsb.tile([P, NPC], DT16, tag="rb16")
                        nc.vector.tensor_copy(rb16[:], rbc[:])
                        nc.vector.tensor_tensor(
                            oS16[:, NPC * hh:NPC * (hh + 1)],
                            o_ps[hh][:], rb16[:], op=A.mult)

            # ---- out_proj^T + residual + LN1^T (all feature-major) ----
            h1T16 = ak.tile([P, 2 * NPC], DT16)
            x1h = ak.tile([P, 2 * NPC], DT16)
            with tc.tile_pool(name="ln_sb", bufs=2) as lsb:
                with tc.tile_pool(name="op_ps", bufs=1, space="PSUM") as ops:
                    x1_ps = [ops.tile([P, NPC], DT32, space="PSUM",
                                      tag=f"x1{c}", name=f"x1{c}")
                             for c in range(2)]
                    for c in range(2):
                        for hh in range(H):
                            nc.tensor.matmul(
                                x1_ps[c][:],
                                lhsT=woT216[:, P * (2 * hh + c):
                                            P * (2 * hh + c + 1)],
                                rhs=oS16[:, NPC * hh:NPC * (hh + 1)],
                                start=(hh == 0), stop=(hh == 1))
                    for c in range(2):
                        nc.vector.scalar_tensor_tensor(
                            x1h[:, NPC * c:NPC * (c + 1)], x1_ps[c][:],
                            cols[:, 2 + c:3 + c],
                            hT16[:, NPC * c:NPC * (c + 1)],
                            op0=A.add, op1=A.add)

                def layernorm_T(dst, xh, gcol, bcol, out_dt, tag):
                    """LN over features (partition dim x 2 chunks), rows via
                    ones-matmuls. xh: [P, 2*NPC] f16. dst written per chunk."""
                    with tc.tile_pool(name=f"ln_ps_{tag}", bufs=1,
                                      space="PSUM") as rws:
                        mu_ps = rws.tile([1, NPC], DT32, space="PSUM",
                                         tag=f"{tag}mu")
                        msq_ps = rws.tile([1, NPC], DT32, space="PSUM",
                                          tag=f"{tag}ms")
                        sq = lsb.tile([P, 2 * NPC], DT16, tag=f"{tag}sq")
                        nc.vector.tensor_tensor(sq[:], xh[:], xh[:], op=A.mult)
                        for c in range(2):
                            nc.tensor.matmul(
                                mu_ps[:], lhsT=ones16c[:],
                                rhs=xh[:, NPC * c:NPC * (c + 1)],
                                start=(c == 0), stop=(c == 1))
                        for c in range(2):
                            nc.tensor.matmul(
                                msq_ps[:], lhsT=ones16c[:],
                                rhs=sq[:, NPC * c:NPC * (c + 1)],
                                start=(c == 0), stop=(c == 1))
                        mu_n = lsb.tile([1, NPC], DT32, tag=f"{tag}mn")
                        nc.vector.tensor_scalar(mu_n[:], mu_ps[:], 1.0 / D,
                                                None, op0=A.mult)
                        nmu16 = lsb.tile([1, NPC], DT16, tag=f"{tag}nm")
                        nc.vector.tensor_scalar(nmu16[:], mu_ps[:], -1.0 / D,
                                                None, op0=A.mult)
                        mu2 = lsb.tile([1, NPC], DT32, tag=f"{tag}m2")
                        nc.vector.tensor_tensor(mu2[:], mu_n[:], mu_n[:],
                                                op=A.mult)
                        var32 = lsb.tile([1, NPC], DT32, tag=f"{tag}vr")
                        nc.vector.scalar_tensor_tensor(
                            var32[:], msq_ps[:], 1.0 / D, mu2[:],
                            op0=A.mult, op1=A.subtract)
                        # rstd via partition-packed sqrt+recip (cheap free dim)
                        packV = rws.tile([P, MPC], DT32, space="PSUM",
                                         tag=f"{tag}pk")
                        for m in range(MPC):
                            nc.tensor.transpose(
                                packV[:, m:m + 1],
                                var32[:, P * m:P * (m + 1)],
                                ident1[:])
                        sdT = lsb.tile([P, MPC], DT32, tag=f"{tag}sd")
                        nc.scalar.activation(sdT[:], packV[:], F.Sqrt,
                                             bias=eps128[:])
                        rstdT16 = lsb.tile([P, MPC], DT16, tag=f"{tag}rs")
                        with nc.allow_low_precision(
                                reason="rstd f16, rel 1e-3 ok"):
                            nc.vector.reciprocal(rstdT16[:], sdT[:])
                        nmu_bc = rws.tile([P, NPC], DT32, space="PSUM",
                                          tag=f"{tag}nb")
                        rstd_bc = rws.tile([P, NPC], DT32, space="PSUM",
                                           tag=f"{tag}rb")
                        nc.tensor.matmul(nmu_bc[:], lhsT=ones16r[:],
                                         rhs=nmu16[:], start=True, stop=True)
                        for m in range(MPC):
                            nc.tensor.matmul(
                                rstd_bc[:, P * m:P * (m + 1)],
                                lhsT=rstdT16[:, m:m + 1].to_broadcast([P, P]),
                                rhs=ident16[:], start=True, stop=True)
                        for c in range(2):
                            t = lsb.tile([P, NPC],
                                         DT16 if out_dt == DT16 else DT32,
                                         tag=f"{tag}t")
                            nc.vector.tensor_tensor(
                                t[:], xh[:, NPC * c:NPC * (c + 1)], nmu_bc[:],
                                op=A.add)
                            t2 = lsb.tile([P, NPC],
                                          DT16 if out_dt == DT16 else DT32,
                                          tag=f"{tag}t2")
                            nc.vector.tensor_tensor(t2[:], t[:], rstd_bc[:],
                                                    op=A.mult)
                            nc.vector.tensor_scalar(
                                dst[:, NPC * c:NPC * (c + 1)], t2[:],
                                gcol[:, c:c + 1], bcol[:, c:c + 1],
                                op0=A.mult, op1=A.add)

                layernorm_T(h1T16, x1h, cols[:, 6:8], cols[:, 8:10], DT16, "a")

                # ---- FFN (fp16, transposed ff2, ff2 interleaved one dc
                #      behind ff1 so relu pipelines under the matmuls) ----
                ff1T = ak.tile([P, (DFF // P) * NPC], DT16)
                x2h = lsb.tile([P, 2 * NPC], DT16, tag="x2h")
                NDC = DFF // P
                with tc.tile_pool(name="f1_ps", bufs=3, space="PSUM") as fps, \
                     tc.tile_pool(name="f2_ps", bufs=1, space="PSUM") as fps2:
                    x2_ps = [fps2.tile([P, NPC], DT32, space="PSUM",
                                       tag=f"x2{c}", name=f"x2{c}")
                             for c in range(2)]

                    def ff2_step(dc):
                        for c in range(2):
                            nc.tensor.matmul(
                                x2_ps[c][:],
                                lhsT=w2T216[:, P * (2 * dc + c):
                                            P * (2 * dc + c + 1)],
                                rhs=ff1T[:, NPC * dc:NPC * (dc + 1)],
                                start=(dc == 0), stop=(dc == NDC - 1))

                    for dc in range(NDC):
                        pf = fps.tile([P, NPC], DT32, space="PSUM", tag="f1")
                        for k in range(2):
                            nc.tensor.matmul(
                                pf[:],
                                lhsT=w1T16[:, DFF * k + P * dc:
                                           DFF * k + P * (dc + 1)],
                                rhs=h1T16[:, NPC * k:NPC * (k + 1)],
                                start=(k == 0), stop=(k == 1))
                        nc.scalar.activation(
                            ff1T[:, NPC * dc:NPC * (dc + 1)], pf[:], F.Relu,
                            bias=b1t[:, dc:dc + 1])
                        if dc >= 1:
                            ff2_step(dc - 1)
                    ff2_step(NDC - 1)
                    for c in range(2):
                        nc.vector.scalar_tensor_tensor(
                            x2h[:, NPC * c:NPC * (c + 1)], x2_ps[c][:],
                            cols[:, 4 + c:5 + c],
                            h1T16[:, NPC * c:NPC * (c + 1)],
                            op0=A.add, op1=A.add)

                out_sb = ak.tile([P, 2 * NPC], DT32)
                layernorm_T(out_sb, x2h, cols[:, 10:12], cols[:, 12:14],
                            DT32, "b")
                nc.scalar.dma_start(out_d[:], out_sb[:])
            ctx_attn.__exit__(None, None, None)

    nc.compile()
    return nc


# ======================= host-side prep =======================

def _prep_inputs(x, edge_index, edge_weight, W_gcn, b_gcn, in_proj_w,
                 in_proj_b, out_proj_w, out_proj_b, lin1_w, lin1_b, lin2_w,
                 lin2_b, ln1_g, ln1_b, ln2_g, ln2_b):
    """Index-permutation / layout prep + edge-weight prenormalization."""
    x = np.asarray(x, np.float32)
    src = np.asarray(edge_index[0], np.int64)
    dst = np.asarray(edge_index[1], np.int64)
    w = np.asarray(edge_weight, np.float64)

    def wrap128(a):
        n = a.shape[0] // P
        return np.ascontiguousarray(
            a.reshape(n, P, a.shape[1]).transpose(1, 0, 2).reshape(P, -1))

    def colsof(v):
        return np.ascontiguousarray(
            np.asarray(v, np.float32).reshape(2, P).T)

    f16 = np.float16
    deg = np.zeros(N, np.float64)
    np.add.at(deg, dst, w)
    deg += 1.0
    dinv = 1.0 / np.sqrt(deg)
    norm = (dinv[src] * w * dinv[dst]).astype(np.float32)

    ipb_np = np.asarray(in_proj_b, np.float32)
    bv = ipb_np[2 * D:]
    bo_eff = (np.asarray(out_proj_w, np.float32) @ bv
              + np.asarray(out_proj_b, np.float32))

    wo = np.asarray(out_proj_w, np.float32)
    woT2 = np.empty((P, 4 * P), np.float32)
    for hh in range(H):
        for c in range(2):
            # lhsT[p, m] = Wo[c*128+m, hh*128+p] / FSC
            woT2[:, P * (2 * hh + c):P * (2 * hh + c + 1)] = \
                wo[c * P:(c + 1) * P, hh * P:(hh + 1) * P].T / FSC

    w2 = np.asarray(lin2_w, np.float32)
    w2T2 = np.empty((P, (DFF // P) * D), np.float32)
    for dc in range(DFF // P):
        for c in range(2):
            w2T2[:, P * (2 * dc + c):P * (2 * dc + c + 1)] = \
                w2[c * P:(c + 1) * P, dc * P:(dc + 1) * P].T

    cols = np.concatenate([
        colsof(b_gcn), colsof(bo_eff), colsof(lin2_b),
        colsof(ln1_g), colsof(ln1_b), colsof(ln2_g), colsof(ln2_b)], axis=1)

    shared = {
        "xTf": wrap128(np.ascontiguousarray(x.T)).astype(f16),
        "wg": wrap128(np.asarray(W_gcn, np.float32)).astype(f16),
        "winT": wrap128(np.ascontiguousarray(
            np.asarray(in_proj_w, np.float32).T)).astype(f16),
        "ipb": np.ascontiguousarray(ipb_np.reshape(6, P).T),
        "woT2": woT2.astype(f16),
        "w1T": wrap128(np.ascontiguousarray(
            np.asarray(lin1_w, np.float32).T)).astype(f16),
        "b1": np.ascontiguousarray(
            np.asarray(lin1_b, np.float32).reshape(DFF // P, P).T),
        "w2T2": w2T2.astype(f16),
        "cols": cols,
        "ident": np.eye(P, dtype=f16),
    }

    core_of = dst // NPC
    in_maps = []
    for c in range(N_CORES):
        sel = np.nonzero(core_of == c)[0]
        s_c = src[sel]
        d_c = (dst[sel] - NPC * c).astype(np.int64)
        n_c = norm[sel]

        w_arr = np.zeros((N, KPAD), np.float32)
        idx_arr = np.full((N, KPAD), -1, np.int16)
        counts = np.zeros(N, np.int32)
        slot_of = {}
        for si, di, wi in zip(s_c.tolist(), d_c.tolist(), n_c.tolist()):
            key = si * NPC + di
            slot = slot_of.get(key)
            if slot is None:
                j = int(counts[si])
                assert j < KPAD, f"KPAD overflow at src {si}"
                counts[si] = j + 1
                w_arr[si, j] = wi
                idx_arr[si, j] = di
                slot_of[key] = j
            else:
                w_arr[si, slot] += wi
        # self loops: weight dinv^2 at (g, g-512c)
        for di in range(NPC):
            g = NPC * c + di
            key = g * NPC + di
            slot = slot_of.get(key)
            if slot is None:
                j = int(counts[g])
                assert j < KPAD, f"KPAD overflow at self {g}"
                counts[g] = j + 1
                w_arr[g, j] = dinv[g] * dinv[g]
                idx_arr[g, j] = di
            else:
                w_arr[g, slot] += dinv[g] * dinv[g]

        in_maps.append({
            **shared,
            "warr": wrap128(w_arr).astype(f16),
            "idx": wrap128(idx_arr),
        })
    return in_maps


# ======================= runner =======================

class _Runner:
    """Persistent-jit SPMD executor (mirrors bass2jax.run_bass_via_pjrt)."""

    def __init__(self, nc):
        import jax
        from jax.sharding import Mesh, PartitionSpec
        from jax.experimental.shard_map import shard_map
        from concourse.bass2jax import (_bass_exec_p, install_neuronx_cc_hook,
                                        partition_id_tensor)
        install_neuronx_cc_hook()
        self.jax = jax
        partition_name = (nc.partition_id_tensor.name
                          if nc.partition_id_tensor else None)
        in_names, out_names, out_avals, zero_outs = [], [], [], []
        for alloc in nc.m.functions[0].allocations:
            if not isinstance(alloc, mybir.MemoryLocationSet):
                continue
            name = alloc.memorylocations[0].name
            if alloc.kind == "ExternalInput":
                if name != partition_name:
                    in_names.append(name)
            elif alloc.kind == "ExternalOutput":
                out_names.append(name)
                shape = tuple(alloc.tensor_shape)
                dtype = mybir.dt.np(alloc.dtype)
                out_avals.append(jax.core.ShapedArray(shape, dtype))
                zero_outs.append(np.zeros(shape, dtype))
        self.in_names, self.out_names = in_names, out_names
        self.out_shapes = [tuple(a.shape) for a in out_avals]
        self.n_params = len(in_names)
        self.zero_outs = zero_outs
        all_in = in_names + out_names
        if partition_name is not None:
            all_in.append(partition_name)

        def _body(*args):
            operands = list(args)
            if partition_name is not None:
                operands.append(partition_id_tensor())
            return tuple(_bass_exec_p.bind(
                *operands, out_avals=tuple(out_avals), in_names=tuple(all_in),
                out_names=tuple(out_names), lowering_input_output_aliases=(),
                sim_require_finite=True, sim_require_nnan=True, nc=nc))

        devices = jax.devices()[:N_CORES]
        self.mesh = Mesh(np.asarray(devices), ("core",))
        nin = self.n_params + len(out_names)
        self.fn = jax.jit(
            shard_map(_body, mesh=self.mesh,
                      in_specs=(PartitionSpec("core"),) * nin,
                      out_specs=(PartitionSpec("core"),) * len(out_names),
                      check_rep=False),
            keep_unused=True)

    def place(self, in_maps):
        import jax
        from jax.sharding import PartitionSpec
        per_core = [[np.asarray(m[n]) for n in self.in_names] for m in in_maps]
        concat = [np.concatenate([per_core[c][i] for c in range(N_CORES)], axis=0)
                  for i in range(self.n_params)]
        zeros = [np.zeros((N_CORES * z.shape[0], *z.shape[1:]), z.dtype)
                 for z in self.zero_outs]
        sh = jax.sharding.NamedSharding(self.mesh, PartitionSpec("core"))
        return [jax.device_put(a, sh) for a in (*concat, *zeros)]

    def run(self, args):
        outs = self.fn(*args)
        self.jax.block_until_ready(outs)
        return outs

    def results(self, outs):
        res = []
        for c in range(N_CORES):
            d = {}
            for i, name in enumerate(self.out_names):
                full = np.asarray(outs[i])
                ps = self.out_shapes[i]
                d[name] = full.reshape((N_CORES,) + ps)[c]
            res.append(d)
        return res


_CACHE = {}


def _get_runner():
    if "runner" not in _CACHE:
        nc = build_kernel()
        _CACHE["nc"] = nc
        _CACHE["runner"] = _Runner(nc)
    return _CACHE["runner"]


def kernel(**inputs) -> np.ndarray:
    runner = _get_runner()
    in_maps = _prep_inputs(**inputs)
    args = runner.place(in_maps)
    outs = runner.run(args)
    res = runner.results(outs)
    # out_d is out^T wrapped: [P, 2, NPC]; out[n, c*128+p] = arr[p, c, n]
    full = np.empty((N, D), np.float32)
    for c in range(N_CORES):
        arr = res[c]["out"].reshape(P, 2, NPC)
        full[NPC * c:NPC * (c + 1)] = arr.transpose(2, 1, 0).reshape(NPC, D)
    return full
